# revision 32
# baseline (speedup 1.0000x reference)
"""Self-contained Trainium2 kernel for nn_DynamicCrossAttention_40286793236903.

kernel(**inputs) takes the FULL inputs (as produced by setup_inputs) and
returns the FULL [4, 256, 64, 64] float32 output.

Sharding: pure data parallel over (batch, image-half): core ci handles
sample b=ci//2, output rows 32*(ci%2)..32*(ci%2)+31. One SPMD Bass program
runs on all 8 cores; all per-core variation is carried in the input data.

Pipeline per core (all feature data bf16 on device):
  1. upsample template 32x32 -> 64x64 (half-pixel bilinear) and build the
     padded combined tensor [512ch, 34, 66] together with the search half.
  2. offsets+mask 3x3 conv as 36 accumulating PE matmuls per col-quarter.
  3. fp32 index math: sample coords, floor/frac, validity-masked bilinear
     weights; a single clamp serves both the y and x gather coordinate.
  4. gather index assembly (PE transposes) into the 16-partition wrapped
     i16 layout dma_gather wants; one index per (tap, pixel) addresses all
     four bilinear corners through the host-staged quad layout
     xq[r] = [pix(r-65), pix(r-1)]  (2KB per descriptor).
  5. per (half, tap): one SWDGE dma_gather (1024 idxs x 2KB), then a fused
     4-op DVE blend chain per 128-pixel block, PE transpose to channel
     major, and a per-half einsum against the deform weights overlapping
     the other half's loop.
"""
import numpy as np
from contextlib import ExitStack

import ml_dtypes
import concourse.bass as bass
import concourse.mybir as mybir
import concourse.tile as tile
from concourse import bacc
from concourse.bass import AP
from concourse.bass_utils import run_bass_kernel_spmd
from concourse.masks import make_identity

F32 = mybir.dt.float32
BF16 = mybir.dt.bfloat16
I32 = mybir.dt.int32
I16 = mybir.dt.int16
ALU = mybir.AluOpType
ACTF = mybir.ActivationFunctionType
BF16NP = ml_dtypes.bfloat16

TAPS = [(ky, kx) for ky in (-1, 0, 1) for kx in (-1, 0, 1)]
C0 = -(15 * 64) - 16 + 1  # idx = 64*Y + X + C0 -> quad row (see xq layout)

_NC_CACHE = {}
LAST_RESULT = None


def build_nc():
    nc = bacc.Bacc(None, target_bir_lowering=False, num_swdge_queues=4)

    tplp = nc.dram_tensor('tplp', [256, 23 * 32], BF16, kind='ExternalInput')
    srch66 = nc.dram_tensor('srch66', [256, 34 * 66], BF16, kind='ExternalInput')
    xq = nc.dram_tensor('xq', [4224, 512], BF16, kind='ExternalInput')
    wpack = nc.dram_tensor('wpack', [128, 4 * 9 * 32], BF16, kind='ExternalInput')
    dwpack = nc.dram_tensor('dwpack', [128, 9 * 2 * 2 * 128], BF16, kind='ExternalInput')
    basei = nc.dram_tensor('basei', [128, 512], F32, kind='ExternalInput')
    mcomb = nc.dram_tensor('mcomb', [128, 4 * 9], F32, kind='ExternalInput')
    bias_comb = nc.dram_tensor('bias_comb', [9, 1], F32, kind='ExternalInput')
    bias_om = nc.dram_tensor('bias_om', [128, 1], F32, kind='ExternalInput')
    bias_out = nc.dram_tensor('bias_out', [256, 1], F32, kind='ExternalInput')
    rmaski = nc.dram_tensor('rmaski', [128, 2], F32, kind='ExternalInput')
    out = nc.dram_tensor('out', [256, 2048], F32, kind='ExternalOutput')

    with tile.TileContext(nc) as tc, ExitStack() as ctx:
        sb = ctx.enter_context(tc.tile_pool(name='sb', bufs=1))
        sbm = ctx.enter_context(tc.tile_pool(name='sbm', bufs=1))
        sbt = ctx.enter_context(tc.tile_pool(name='sbt', bufs=2))
        sba = ctx.enter_context(tc.tile_pool(name='sba', bufs=3))
        gpool2 = ctx.enter_context(tc.tile_pool(name='gpool2', bufs=4))
        spool = ctx.enter_context(tc.tile_pool(name='spool', bufs=2))
        stpool = ctx.enter_context(tc.tile_pool(name='stpool', bufs=3))

        ident = sb.tile([128, 128], BF16, tag='ident', name='ident')
        make_identity(nc, ident[:])
        identf = sb.tile([128, 128], F32, tag='identf', name='identf')
        make_identity(nc, identf[:])

        wp = sb.tile([128, 4 * 9 * 32], BF16, tag='wp', name='wp')
        dw = sb.tile([128, 9 * 2 * 2 * 128], BF16, tag='dw', name='dw')
        base_sb = sb.tile([128, 512], F32, tag='base', name='base')
        nc.sync.dma_start(base_sb[:], basei[:])
        mcomb_sb = sb.tile([128, 4 * 9], F32, tag='mcomb', name='mcomb')
        nc.sync.dma_start(mcomb_sb[:], mcomb[:])
        bcomb_sb = sb.tile([9, 1], F32, tag='bcomb', name='bcomb')
        nc.sync.dma_start(bcomb_sb[:], bias_comb[:])
        bom_sb = sb.tile([128, 1], F32, tag='bom', name='bom')
        nc.sync.dma_start(bom_sb[:], bias_om[:])
        bout_sb = sb.tile([128, 2], F32, tag='bout', name='bout')
        nc.sync.dma_start(bout_sb[:], bias_out[:].rearrange('(g p) o -> p (g o)', g=2))
        rmask_sb = sb.tile([128, 2], F32, tag='rmask', name='rmask')
        nc.sync.dma_start(rmask_sb[:], rmaski[:])

        # ---- stage 1: upsample template + build combined [512ch, 34, 66] ----
        with tc.tile_pool(name='convsb', bufs=1) as convsb, \
             tc.tile_pool(name='psA', bufs=1, space='PSUM') as psA:
            tps = []
            for cg in range(2):
                tp = convsb.tile([128, 23 * 32], BF16, tag=f'tp{cg}', name=f'tp{cg}')
                nc.sync.dma_start(tp[:], tplp[128 * cg:128 * (cg + 1), :])
                tps.append(tp)
            scbs = []
            for cg in range(2):
                cb = convsb.tile([128, 34 * 66], BF16, tag=f'comb{cg+2}', name=f'comb{cg+2}')
                nc.sync.dma_start(cb[:], srch66[128 * cg:128 * (cg + 1), :])
                scbs.append(cb)
            nc.sync.dma_start(wp[:], wpack[:])
            comb = []
            for cg in range(2):
                tp = tps[cg]
                tp3 = tp[:].rearrange('p (r w) -> p r w', r=23)
                V = convsb.tile([128, 34 * 32], BF16, tag=f'vt{cg}', name=f'vt{cg}')
                V3 = V[:].rearrange('p (r w) -> p r w', r=34)
                tmp = convsb.tile([128, 34 * 32], BF16, tag=f'ut{cg}', name=f'ut{cg}')
                tmp3 = tmp[:].rearrange('p (r w) -> p r w', r=34)
                # vertical: V[i] = wa*tp[j] + wb*tp[j+1] (ts 4x + tt 2x; no stt)
                nc.vector.tensor_scalar_mul(tmp3[:, 0:16, :], tp3[:, 2:18, :], 0.25)
                nc.scalar.activation(V3[:, 2:34:2, :], tp3[:, 1:17, :], ACTF.Identity, scale=0.75)
                nc.vector.tensor_tensor(V3[:, 2:34:2, :], V3[:, 2:34:2, :], tmp3[:, 0:16, :], ALU.add)
                nc.vector.tensor_scalar_mul(tmp3[:, 0:16, :], tp3[:, 1:17, :], 0.75)
                nc.scalar.activation(V3[:, 1:33:2, :], tp3[:, 0:16, :], ACTF.Identity, scale=0.25)
                nc.vector.tensor_tensor(V3[:, 1:33:2, :], V3[:, 1:33:2, :], tmp3[:, 0:16, :], ALU.add)
                nc.vector.tensor_scalar_mul(tmp3[:, 0:1, :], tp3[:, 20:21, :], 0.25)
                nc.scalar.activation(V3[:, 0:1, :], tp3[:, 19:20, :], ACTF.Identity, scale=0.75)
                nc.vector.tensor_tensor(V3[:, 0:1, :], V3[:, 0:1, :], tmp3[:, 0:1, :], ALU.add)
                nc.vector.tensor_scalar_mul(tmp3[:, 0:1, :], tp3[:, 22:23, :], 0.75)
                nc.scalar.activation(V3[:, 33:34, :], tp3[:, 21:22, :], ACTF.Identity, scale=0.25)
                nc.vector.tensor_tensor(V3[:, 33:34, :], V3[:, 33:34, :], tmp3[:, 0:1, :], ALU.add)
                cb = convsb.tile([128, 34 * 66], BF16, tag=f'comb{cg}', name=f'comb{cg}')
                cb3 = cb[:].rearrange('p (r w) -> p r w', r=34)
                nc.vector.memset(cb[:], 0.0)
                h3 = tmp3
                nc.vector.tensor_scalar_mul(h3[:, :, 0:31], V3[:, :, 1:32], 0.75)
                nc.scalar.activation(cb3[:, :, 3:65:2], V3[:, :, 0:31], ACTF.Identity, scale=0.25)
                nc.vector.tensor_tensor(cb3[:, :, 3:65:2], cb3[:, :, 3:65:2], h3[:, :, 0:31], ALU.add)
                nc.vector.tensor_scalar_mul(h3[:, :, 0:31], V3[:, :, 1:32], 0.25)
                nc.scalar.activation(cb3[:, :, 2:64:2], V3[:, :, 0:31], ACTF.Identity, scale=0.75)
                nc.vector.tensor_tensor(cb3[:, :, 2:64:2], cb3[:, :, 2:64:2], h3[:, :, 0:31], ALU.add)
                nc.vector.tensor_copy(cb3[:, :, 1:2], V3[:, :, 0:1])
                nc.vector.tensor_copy(cb3[:, :, 64:65], V3[:, :, 31:32])
                comb.append(cb)
            comb += scbs

            # ---- stage 2: offsets+mask conv, col-tiled quarters ----
            wp4 = wp[:].rearrange('p (g t m) -> p g t m', g=4, t=9)
            pom = psA.tile([128, 512], F32, name='pom')
            for q in range(4):
                first = True
                for gi, g in enumerate((2, 3, 0, 1)):
                    cb3 = comb[g][:].rearrange('p (r w) -> p r w', r=34)
                    for t, (ky, kx) in enumerate(TAPS):
                        rhs = cb3[:, 8 * q + 1 + ky: 8 * q + 9 + ky, 1 + kx: 65 + kx]
                        nc.tensor.matmul(
                            pom[32 * q:32 * q + 32, :], wp4[:, g, t, :], rhs,
                            start=first, stop=(gi == 3 and t == 8),
                            tile_position=(0, 32 * q))
                        first = False
            om = sb.tile([128, 512], F32, tag='om', name='om')
            nc.scalar.activation(om[:], pom[:], ACTF.Identity, bias=bom_sb[:], scale=1.0)

        sg = sb.tile([128, 512], F32, tag='sg', name='sg')
        nc.scalar.activation(sg[:], om[:], ACTF.Sigmoid)

        # ---- stage 3: index math (fp32, in-place tile reuse) ----
        def mtile(tag, dt=F32):
            return sbm.tile([128, 512], dt, tag=tag, name=tag)
        P = mtile('P')          # becomes Wf
        nc.vector.tensor_tensor(P[:], om[:], base_sb[:], ALU.add)
        nc.vector.tensor_scalar(P[:], P[:], 96.5, 14.0, ALU.min, ALU.max)
        T32 = mtile('T32', I32)
        nc.vector.tensor_copy(T32[:], P[:])
        Tf = mtile('Tf')        # becomes F (floor)
        nc.vector.tensor_copy(Tf[:], T32[:])
        Gg = mtile('Gg')        # becomes V0
        nc.vector.tensor_tensor(Gg[:], Tf[:], P[:], ALU.is_gt)
        nc.vector.tensor_tensor(Tf[:], Tf[:], Gg[:], ALU.subtract)
        nc.vector.tensor_tensor(P[:], P[:], Tf[:], ALU.subtract)
        Ff, Wf = Tf, P
        Vt = mtile('Vt')
        V0 = Gg
        nc.vector.tensor_scalar(V0[:], Ff[:], 16.0, None, ALU.is_ge)
        nc.vector.tensor_scalar(Vt[:], Ff[:], 80.0, None, ALU.is_lt)
        nc.vector.tensor_tensor(V0[:], V0[:], Vt[:], ALU.mult)
        V1 = mtile('V1')
        nc.vector.tensor_scalar(V1[:], Ff[:], 15.0, None, ALU.is_ge)
        nc.vector.tensor_scalar(Vt[:], Ff[:], 79.0, None, ALU.is_lt)
        nc.vector.tensor_tensor(V1[:], V1[:], Vt[:], ALU.mult)
        W0 = mtile('W0')
        nc.vector.tensor_scalar(W0[:], Wf[:], -1.0, 1.0, ALU.mult, ALU.add)
        nc.vector.tensor_tensor(W0[:], W0[:], V0[:], ALU.mult)
        W1 = Wf
        nc.vector.tensor_tensor(W1[:], Wf[:], V1[:], ALU.mult)
        # single gather coordinate: clamp(floor, 15, 79) serves y and x rows
        AxC = V1
        nc.vector.tensor_scalar(AxC[:], Ff[:], 79.0, 15.0, ALU.min, ALU.max)

        # ---- stage 4: idx assembly -> idxf [9, 2048] f32 (pixel-major) ----
        mc4 = mcomb_sb[:].rearrange('p (q m) -> p q m', q=4)
        idxf = sb.tile([9, 2048], F32, tag='idxf', name='idxf')
        with tc.tile_pool(name='psI', bufs=2, space='PSUM') as psI:
            for q in range(4):
                pidx = psI.tile([9, 512], F32, name='pidx')
                nc.tensor.matmul(pidx[:], mc4[:, q, :], AxC[:], start=True, stop=True)
                nc.scalar.activation(idxf[:, 512 * q:512 * (q + 1)], pidx[:],
                                      ACTF.Identity, bias=bcomb_sb[:], scale=1.0)
        # wrap for dma_gather, grouped so a 3-tap batch has one contiguous
        # idx window: idx16[j, 576*hb + 64*t + 8*bbl + a] = idxf[t, 128*(8*hb+bbl) + 16*a + j]
        idx16 = sb.tile([128, 2 * 9 * 64], I16, tag='idx16', name='idx16')
        tsb = sb.tile([128, 16 * 9], F32, tag='tsb', name='tsb')
        tsb3 = tsb[:].rearrange('p (b t) -> p b t', b=16)
        with tc.tile_pool(name='psT', bufs=2, space='PSUM') as psT:
            for bb in range(16):
                pT = psT.tile([128, 9], F32, name='pT')
                nc.tensor.transpose(pT[:], idxf[:, 128 * bb:128 * (bb + 1)],
                                    identf[0:9, 0:9], tile_position=(0, 0))
                nc.vector.tensor_copy(tsb3[:, bb, :], pT[:])
            for a in range(8):
                pW = psT.tile([16, 144], F32, name='pW')
                nc.tensor.matmul(pW[:], identf[:, 16 * a:16 * (a + 1)], tsb[:],
                                 start=True, stop=True)
                dsta = AP(idx16[:].tensor, idx16[:].offset + a,
                          [[9 * 128, 16], [576, 2], [8, 8], [64, 9]])
                nc.vector.tensor_copy(
                    dsta, pW[:].rearrange('p (h b t) -> p h b t', h=2, b=8))
        nc.sync.dma_start(idx16[16:32, :], idx16[0:16, :])
        nc.sync.dma_start(idx16[32:64, :], idx16[0:32, :])
        nc.sync.dma_start(idx16[64:128, :], idx16[0:64, :])

        # ---- blend weights: mask-select, transpose, products ----
        for Wt in (W0, W1):
            nc.vector.tensor_scalar(Wt[:], Wt[:], rmask_sb[:, 0:1], None, ALU.mult)
            nc.vector.scalar_tensor_tensor(Wt[:], sg[:], rmask_sb[:, 1:2], Wt[:],
                                           ALU.mult, ALU.add)
        wprod = sb.tile([128, 16 * 6 * 9], F32, tag='wprod', name='wprod')
        wp3 = wprod[:].rearrange('p (b s t) -> p b s t', b=16, s=6)
        with tc.tile_pool(name='psW', bufs=2, space='PSUM') as psW:
            for b in range(16):
                q, cc = b // 4, b % 4
                pt = psW.tile([128, 54], F32, name='ptw')
                idq = identf[32 * q:32 * q + 27, 32 * q:32 * q + 27]
                nc.tensor.transpose(pt[:, 0:27], W0[32 * q:32 * q + 27, 128 * cc:128 * (cc + 1)],
                                    idq, tile_position=(32 * q, 0))
                nc.tensor.transpose(pt[:, 27:54], W1[32 * q:32 * q + 27, 128 * cc:128 * (cc + 1)],
                                    idq, tile_position=(32 * q, 0))
                ta = sbt.tile([128, 54], F32, tag='tall', name='tall')
                nc.vector.tensor_copy(ta[:], pt[:])
                r0 = sbt.tile([128, 9], F32, tag='r0', name='r0')
                nc.vector.tensor_tensor(r0[:], ta[:, 0:9], ta[:, 18:27], ALU.mult)
                r1 = sbt.tile([128, 9], F32, tag='r1', name='r1')
                nc.vector.tensor_tensor(r1[:], ta[:, 27:36], ta[:, 45:54], ALU.mult)
                nc.vector.tensor_tensor(wp3[:, b, 0, :], r0[:], ta[:, 9:18], ALU.mult)
                nc.vector.tensor_tensor(wp3[:, b, 1, :], r0[:], ta[:, 36:45], ALU.mult)
                nc.vector.tensor_tensor(wp3[:, b, 2, :], r1[:], ta[:, 9:18], ALU.mult)
                nc.vector.tensor_tensor(wp3[:, b, 3, :], r1[:], ta[:, 36:45], ALU.mult)

        # ---- stages 5-8: per (half, tap) gather -> blend -> transpose ----
        # einsum accumulates in PSUM inside the loop (delayed one tap so PE
        # never waits on the PSUM->SBUF staging copies)
        nc.sync.dma_start(dw[:], dwpack[:])
        gsems = [nc.alloc_semaphore(f'gsem{q}') for q in range(4)]
        gcnt = [0, 0, 0, 0]
        inap = AP(xq[:].tensor, 0, [[512, 4223], [1, 1024]])
        dw4 = dw[:].rearrange('p (k g o c) -> p k g o c', k=9, g=2, o=2)
        with tc.tile_pool(name='psQ', bufs=2, space='PSUM') as psQ, \
             tc.tile_pool(name='psO', bufs=1, space='PSUM') as psO:
            for hb in range(2):
                po = [psO.tile([128, 512], F32, name=f'po{og}{qq}')
                      for og in range(2) for qq in range(2)]

                def einsum_tap(t, stg3):
                    for og in range(2):
                        for qq in range(2):
                            for cg in range(2):
                                nc.tensor.matmul(
                                    po[2 * og + qq][:], dw4[:, t, cg, og, :],
                                    stg3[:, cg, 512 * qq:512 * (qq + 1)],
                                    start=(t == 0 and cg == 0),
                                    stop=(t == 8 and cg == 1))

                prev = None
                gb = None
                gstart = {t: (t, 1) for t in range(9)}
                for t in range(9):
                    if t in gstart:
                        tb, ntap = gstart[t][0], gstart[t][1]
                        gb = gpool2.tile([128, 8 * ntap, 1024], BF16,
                                         tag=f'gq{ntap}', name=f'gq{ntap}')
                        qn = (9 * hb + tb) % 4
                        c0 = 576 * hb + 64 * t
                        nc.gpsimd.dma_gather(
                            out_ap=gb[:], in_ap=inap,
                            idxs_ap=idx16[:, c0: c0 + 64 * ntap],
                            num_idxs=1024 * ntap, num_idxs_reg=1024 * ntap,
                            elem_size=1024, elem_step=512, queue_num=qn,
                            prepare_only=True, sem=gsems[qn])
                        nc.gpsimd.trigger_dma(count=None, queue_num=qn)
                        gcnt[qn] += 1
                        nc.vector.wait_ge(gsems[qn], 16 * gcnt[qn])
                        nc.scalar.wait_ge(gsems[qn], 16 * gcnt[qn])
                        gt0 = t
                    S = spool.tile([128, 8 * 256], BF16, tag='S', name='S')
                    S3 = S[:].rearrange('p (b n) -> p b n', b=8)
                    for blk in range(8):
                        b = 8 * hb + blk
                        gq4 = gb[:]
                        bl = 8 * (t - gt0) + blk
                        m = sba.tile([128, 256], BF16, tag='m', name='m')
                        m2 = sba.tile([128, 256], BF16, tag='m2', name='m2')
                        a1 = sba.tile([128, 256], BF16, tag='a1', name='a1')
                        a2 = sba.tile([128, 256], BF16, tag='a2', name='a2')
                        nc.vector.tensor_scalar_mul(m[:], gq4[:, bl, 0:256],
                                                    wp3[:, b, 0, t:t + 1])
                        nc.scalar.activation(a1[:], gq4[:, bl, 256:512], ACTF.Identity,
                                             scale=wp3[:, b, 2, t:t + 1])
                        nc.vector.tensor_scalar_mul(m2[:], gq4[:, bl, 512:768],
                                                    wp3[:, b, 1, t:t + 1])
                        nc.scalar.activation(a2[:], gq4[:, bl, 768:1024], ACTF.Identity,
                                             scale=wp3[:, b, 3, t:t + 1])
                        nc.vector.tensor_tensor(m[:], m[:], m2[:], ALU.add)
                        nc.vector.tensor_tensor(a1[:], a1[:], a2[:], ALU.add)
                        nc.vector.tensor_tensor(S3[:, blk, :], m[:], a1[:], ALU.add)
                    stg = stpool.tile([128, 2, 1024], BF16, tag='stg', name='stg')
                    for half in range(2):
                        pq = psQ.tile([128, 1024], BF16, name='pq')
                        for j in range(4):
                            blk = 4 * half + j
                            for cg in range(2):
                                nc.tensor.transpose(
                                    pq[:, 256 * j + 128 * cg: 256 * j + 128 * (cg + 1)],
                                    S3[:, blk, 128 * cg:128 * (cg + 1)], ident[:])
                        pq4 = pq[:].rearrange('p (j g c) -> p j g c', j=4, g=2)
                        for cg in range(2):
                            dstp = stg[:, cg, 512 * half:512 * (half + 1)]
                            if half == 0:
                                nc.scalar.activation(
                                    dstp.rearrange('p (j c) -> p j c', j=4),
                                    pq4[:, :, cg, :], ACTF.Identity)
                            else:
                                nc.vector.tensor_copy(
                                    dstp.rearrange('p (j c) -> p j c', j=4),
                                    pq4[:, :, cg, :])
                    if prev is not None:
                        einsum_tap(prev[0], prev[1])
                    prev = (t, stg[:])
                einsum_tap(prev[0], prev[1])
                # ---- bias + store for this half ----
                for og in range(2):
                    for qq in range(2):
                        q = 2 * hb + qq
                        osb = sbt.tile([128, 512], F32, tag='osb', name='osb')
                        nc.scalar.activation(osb[:], po[2 * og + qq][:], ACTF.Identity,
                                             bias=bout_sb[:, og:og + 1], scale=1.0)
                        nc.sync.dma_start(out[128 * og:128 * (og + 1), 512 * q:512 * (q + 1)],
                                          osb[:])

    nc.compile()
    return nc


def prep_core_inputs(inputs, b, h):
    tf = np.ascontiguousarray(np.asarray(inputs['template_feat'][b], dtype=np.float32))
    sf = np.ascontiguousarray(np.asarray(inputs['search_feat'][b], dtype=np.float32))
    offset_w = np.asarray(inputs['offset_w'], dtype=np.float32)
    offset_b = np.asarray(inputs['offset_b'], dtype=np.float32)
    mask_w = np.asarray(inputs['mask_w'], dtype=np.float32)
    mask_b = np.asarray(inputs['mask_b'], dtype=np.float32)
    deform_w = np.asarray(inputs['deform_w'], dtype=np.float32)
    deform_b = np.asarray(inputs['deform_b'], dtype=np.float32)

    tplp = np.zeros((256, 23, 32), np.float32)
    for j in range(19):
        tplp[:, j] = tf[:, min(max(16 * h - 1 + j, 0), 31)]
    if h == 0:
        tplp[:, 21] = tf[:, 15]
        tplp[:, 22] = tf[:, 16]
    else:
        tplp[:, 19] = tf[:, 15]
        tplp[:, 20] = tf[:, 16]

    srch66 = np.zeros((256, 34, 66), np.float32)
    for i in range(34):
        r = 32 * h - 1 + i
        if 0 <= r <= 63:
            srch66[:, i, 1:65] = sf[:, r]

    # quad layout: xq[r] = [pix(r-65), pix(r-1)]; one idx r fetches
    # rows r, r+1 = [TL, BL, TR, BR] corners (2KB).
    sfp = sf.reshape(256, 4096).T  # [4096 px, 256 ch]
    xquad = np.zeros((4224, 512), np.float32)
    xquad[65:65 + 4096, 0:256] = sfp
    xquad[1:1 + 4096, 256:512] = sfp

    wpack = np.zeros((128, 4, 9, 32), np.float32)
    for g in range(4):
        for t, (ky, kx) in enumerate(TAPS):
            cs = slice(128 * g, 128 * (g + 1))
            wpack[:, g, t, 0:9] = offset_w[0::2, cs, ky + 1, kx + 1].T
            wpack[:, g, t, 9:18] = offset_w[1::2, cs, ky + 1, kx + 1].T
            if ky == 0 and kx == 0:
                wpack[:, g, t, 18:27] = mask_w[:, cs, 0, 0].T
    wk = deform_w.reshape(256, 256, 3, 3)
    dwp = np.zeros((128, 9, 2, 2, 128), np.float32)
    for t in range(9):
        ky, kx = TAPS[t]
        for cg in range(2):
            for og in range(2):
                dwp[:, t, cg, og, :] = wk[128 * og:128 * (og + 1),
                                          128 * cg:128 * (cg + 1), ky + 1, kx + 1].T

    basei = np.zeros((128, 512), np.float32)
    col = np.arange(512)
    for q in range(4):
        for m in range(9):
            basei[32 * q + m] = 32 * h + 8 * q + col // 64 + TAPS[m][0] + 16
            basei[32 * q + 9 + m] = col % 64 + TAPS[m][1] + 16

    mcomb = np.zeros((128, 4, 9), np.float32)
    for q in range(4):
        for t in range(9):
            mcomb[32 * q + t, q, t] = 64.0
            mcomb[32 * q + 9 + t, q, t] = 1.0
    bias_comb = np.full((9, 1), float(C0), np.float32)

    bias_om = np.zeros((128, 1), np.float32)
    for q in range(4):
        bias_om[32 * q + 0:32 * q + 9, 0] = offset_b[0::2]
        bias_om[32 * q + 9:32 * q + 18, 0] = offset_b[1::2]
        bias_om[32 * q + 18:32 * q + 27, 0] = mask_b

    rmaski = np.zeros((128, 2), np.float32)
    for q in range(4):
        rmaski[32 * q:32 * q + 18, 0] = 1.0
        rmaski[32 * q + 18:32 * q + 32, 1] = 1.0

    return {
        'rmaski': rmaski,
        'tplp': tplp.reshape(256, 23 * 32).astype(BF16NP),
        'srch66': srch66.reshape(256, 34 * 66).astype(BF16NP),
        'xq': xquad.astype(BF16NP),
        'wpack': wpack.reshape(128, 4 * 9 * 32).astype(BF16NP),
        'dwpack': dwp.reshape(128, 9 * 2 * 2 * 128).astype(BF16NP),
        'basei': basei,
        'mcomb': mcomb.reshape(128, 4 * 9),
        'bias_comb': bias_comb,
        'bias_om': bias_om,
        'bias_out': deform_b.reshape(256, 1).astype(np.float32),
    }


def kernel(**inputs):
    key = 'v2'
    if key not in _NC_CACHE:
        _NC_CACHE[key] = build_nc()
    nc = _NC_CACHE[key]
    in_maps = [prep_core_inputs(inputs, ci // 2, ci % 2) for ci in range(8)]
    res = run_bass_kernel_spmd(nc, in_maps, core_ids=list(range(8)))
    global LAST_RESULT
    LAST_RESULT = res
    out = np.zeros((4, 256, 64, 64), np.float32)
    for ci in range(8):
        b, h = ci // 2, ci % 2
        out[b][:, 32 * h:32 * h + 32, :] = res.results[ci]['out'].reshape(256, 32, 64)
    return out


# revision 33
# speedup vs baseline: 1.0043x; 1.0043x over previous
"""Self-contained Trainium2 kernel for nn_DynamicCrossAttention_40286793236903.

kernel(**inputs) takes the FULL inputs (as produced by setup_inputs) and
returns the FULL [4, 256, 64, 64] float32 output.

Sharding: pure data parallel over (batch, image-half): core ci handles
sample b=ci//2, output rows 32*(ci%2)..32*(ci%2)+31. One SPMD Bass program
runs on all 8 cores; all per-core variation is carried in the input data.

Pipeline per core (all feature data bf16 on device):
  1. upsample template 32x32 -> 64x64 (half-pixel bilinear) and build the
     padded combined tensor [512ch, 34, 66] together with the search half.
  2. offsets+mask 3x3 conv as 36 accumulating PE matmuls per col-quarter.
  3. fp32 index math: sample coords, floor/frac, validity-masked bilinear
     weights; a single clamp serves both the y and x gather coordinate.
  4. gather index assembly (PE transposes) into the 16-partition wrapped
     i16 layout dma_gather wants; one index per (tap, pixel) addresses all
     four bilinear corners through the host-staged quad layout
     xq[r] = [pix(r-65), pix(r-1)]  (2KB per descriptor).
  5. per (half, tap): one SWDGE dma_gather (1024 idxs x 2KB), then a fused
     4-op DVE blend chain per 128-pixel block, PE transpose to channel
     major, and a per-half einsum against the deform weights overlapping
     the other half's loop.
"""
import numpy as np
from contextlib import ExitStack

import ml_dtypes
import concourse.bass as bass
import concourse.mybir as mybir
import concourse.tile as tile
from concourse import bacc
from concourse.bass import AP
from concourse.bass_utils import run_bass_kernel_spmd
from concourse.masks import make_identity

F32 = mybir.dt.float32
BF16 = mybir.dt.bfloat16
I32 = mybir.dt.int32
I16 = mybir.dt.int16
ALU = mybir.AluOpType
ACTF = mybir.ActivationFunctionType
BF16NP = ml_dtypes.bfloat16

TAPS = [(ky, kx) for ky in (-1, 0, 1) for kx in (-1, 0, 1)]
C0 = -(15 * 64) - 16 + 1  # idx = 64*Y + X + C0 -> quad row (see xq layout)

_NC_CACHE = {}
LAST_RESULT = None


def build_nc():
    nc = bacc.Bacc(None, target_bir_lowering=False, num_swdge_queues=4)

    tplp = nc.dram_tensor('tplp', [256, 23 * 32], BF16, kind='ExternalInput')
    srch66 = nc.dram_tensor('srch66', [256, 34 * 66], BF16, kind='ExternalInput')
    xq = nc.dram_tensor('xq', [4224, 512], BF16, kind='ExternalInput')
    wpack = nc.dram_tensor('wpack', [128, 4 * 9 * 32], BF16, kind='ExternalInput')
    dwpack = nc.dram_tensor('dwpack', [128, 9 * 2 * 2 * 128], BF16, kind='ExternalInput')
    basei = nc.dram_tensor('basei', [128, 512], F32, kind='ExternalInput')
    mcomb = nc.dram_tensor('mcomb', [128, 4 * 9], F32, kind='ExternalInput')
    bias_comb = nc.dram_tensor('bias_comb', [9, 1], F32, kind='ExternalInput')
    bias_om = nc.dram_tensor('bias_om', [128, 1], F32, kind='ExternalInput')
    bias_out = nc.dram_tensor('bias_out', [256, 1], F32, kind='ExternalInput')
    rmaski = nc.dram_tensor('rmaski', [128, 2], F32, kind='ExternalInput')
    out = nc.dram_tensor('out', [256, 2048], F32, kind='ExternalOutput')

    with tile.TileContext(nc) as tc, ExitStack() as ctx:
        sb = ctx.enter_context(tc.tile_pool(name='sb', bufs=1))
        sbm = ctx.enter_context(tc.tile_pool(name='sbm', bufs=1))
        sbt = ctx.enter_context(tc.tile_pool(name='sbt', bufs=2))
        sba = ctx.enter_context(tc.tile_pool(name='sba', bufs=3))
        gpool2 = ctx.enter_context(tc.tile_pool(name='gpool2', bufs=5))
        spool = ctx.enter_context(tc.tile_pool(name='spool', bufs=2))
        stpool = ctx.enter_context(tc.tile_pool(name='stpool', bufs=3))

        ident = sb.tile([128, 128], BF16, tag='ident', name='ident')
        make_identity(nc, ident[:])
        identf = sb.tile([128, 128], F32, tag='identf', name='identf')
        make_identity(nc, identf[:])

        wp = sb.tile([128, 4 * 9 * 32], BF16, tag='wp', name='wp')
        dw = sb.tile([128, 9 * 2 * 2 * 128], BF16, tag='dw', name='dw')
        base_sb = sb.tile([128, 512], F32, tag='base', name='base')
        nc.sync.dma_start(base_sb[:], basei[:])
        mcomb_sb = sb.tile([128, 4 * 9], F32, tag='mcomb', name='mcomb')
        nc.sync.dma_start(mcomb_sb[:], mcomb[:])
        bcomb_sb = sb.tile([9, 1], F32, tag='bcomb', name='bcomb')
        nc.sync.dma_start(bcomb_sb[:], bias_comb[:])
        bom_sb = sb.tile([128, 1], F32, tag='bom', name='bom')
        nc.sync.dma_start(bom_sb[:], bias_om[:])
        bout_sb = sb.tile([128, 2], F32, tag='bout', name='bout')
        nc.sync.dma_start(bout_sb[:], bias_out[:].rearrange('(g p) o -> p (g o)', g=2))
        rmask_sb = sb.tile([128, 2], F32, tag='rmask', name='rmask')
        nc.sync.dma_start(rmask_sb[:], rmaski[:])

        # ---- stage 1: upsample template + build combined [512ch, 34, 66] ----
        with tc.tile_pool(name='convsb', bufs=1) as convsb, \
             tc.tile_pool(name='psA', bufs=1, space='PSUM') as psA:
            tps = []
            for cg in range(2):
                tp = convsb.tile([128, 23 * 32], BF16, tag=f'tp{cg}', name=f'tp{cg}')
                nc.sync.dma_start(tp[:], tplp[128 * cg:128 * (cg + 1), :])
                tps.append(tp)
            scbs = []
            for cg in range(2):
                cb = convsb.tile([128, 34 * 66], BF16, tag=f'comb{cg+2}', name=f'comb{cg+2}')
                nc.sync.dma_start(cb[:], srch66[128 * cg:128 * (cg + 1), :])
                scbs.append(cb)
            nc.sync.dma_start(wp[:], wpack[:])
            comb = []
            for cg in range(2):
                tp = tps[cg]
                tp3 = tp[:].rearrange('p (r w) -> p r w', r=23)
                V = convsb.tile([128, 34 * 32], BF16, tag=f'vt{cg}', name=f'vt{cg}')
                V3 = V[:].rearrange('p (r w) -> p r w', r=34)
                tmp = convsb.tile([128, 34 * 32], BF16, tag=f'ut{cg}', name=f'ut{cg}')
                tmp3 = tmp[:].rearrange('p (r w) -> p r w', r=34)
                # vertical: V[i] = wa*tp[j] + wb*tp[j+1] (ts 4x + tt 2x; no stt)
                nc.vector.tensor_scalar_mul(tmp3[:, 0:16, :], tp3[:, 2:18, :], 0.25)
                nc.scalar.activation(V3[:, 2:34:2, :], tp3[:, 1:17, :], ACTF.Identity, scale=0.75)
                nc.vector.tensor_tensor(V3[:, 2:34:2, :], V3[:, 2:34:2, :], tmp3[:, 0:16, :], ALU.add)
                nc.vector.tensor_scalar_mul(tmp3[:, 0:16, :], tp3[:, 1:17, :], 0.75)
                nc.scalar.activation(V3[:, 1:33:2, :], tp3[:, 0:16, :], ACTF.Identity, scale=0.25)
                nc.vector.tensor_tensor(V3[:, 1:33:2, :], V3[:, 1:33:2, :], tmp3[:, 0:16, :], ALU.add)
                nc.vector.tensor_scalar_mul(tmp3[:, 0:1, :], tp3[:, 20:21, :], 0.25)
                nc.scalar.activation(V3[:, 0:1, :], tp3[:, 19:20, :], ACTF.Identity, scale=0.75)
                nc.vector.tensor_tensor(V3[:, 0:1, :], V3[:, 0:1, :], tmp3[:, 0:1, :], ALU.add)
                nc.vector.tensor_scalar_mul(tmp3[:, 0:1, :], tp3[:, 22:23, :], 0.75)
                nc.scalar.activation(V3[:, 33:34, :], tp3[:, 21:22, :], ACTF.Identity, scale=0.25)
                nc.vector.tensor_tensor(V3[:, 33:34, :], V3[:, 33:34, :], tmp3[:, 0:1, :], ALU.add)
                cb = convsb.tile([128, 34 * 66], BF16, tag=f'comb{cg}', name=f'comb{cg}')
                cb3 = cb[:].rearrange('p (r w) -> p r w', r=34)
                nc.vector.memset(cb[:], 0.0)
                h3 = tmp3
                nc.vector.tensor_scalar_mul(h3[:, :, 0:31], V3[:, :, 1:32], 0.75)
                nc.scalar.activation(cb3[:, :, 3:65:2], V3[:, :, 0:31], ACTF.Identity, scale=0.25)
                nc.vector.tensor_tensor(cb3[:, :, 3:65:2], cb3[:, :, 3:65:2], h3[:, :, 0:31], ALU.add)
                nc.vector.tensor_scalar_mul(h3[:, :, 0:31], V3[:, :, 1:32], 0.25)
                nc.scalar.activation(cb3[:, :, 2:64:2], V3[:, :, 0:31], ACTF.Identity, scale=0.75)
                nc.vector.tensor_tensor(cb3[:, :, 2:64:2], cb3[:, :, 2:64:2], h3[:, :, 0:31], ALU.add)
                nc.vector.tensor_copy(cb3[:, :, 1:2], V3[:, :, 0:1])
                nc.vector.tensor_copy(cb3[:, :, 64:65], V3[:, :, 31:32])
                comb.append(cb)
            comb += scbs

            # ---- stage 2: offsets+mask conv, col-tiled quarters ----
            wp4 = wp[:].rearrange('p (g t m) -> p g t m', g=4, t=9)
            pom = psA.tile([128, 512], F32, name='pom')
            for q in range(4):
                first = True
                for gi, g in enumerate((2, 3, 0, 1)):
                    cb3 = comb[g][:].rearrange('p (r w) -> p r w', r=34)
                    for t, (ky, kx) in enumerate(TAPS):
                        rhs = cb3[:, 8 * q + 1 + ky: 8 * q + 9 + ky, 1 + kx: 65 + kx]
                        nc.tensor.matmul(
                            pom[32 * q:32 * q + 32, :], wp4[:, g, t, :], rhs,
                            start=first, stop=(gi == 3 and t == 8),
                            tile_position=(0, 32 * q))
                        first = False
            om = sb.tile([128, 512], F32, tag='om', name='om')
            nc.scalar.activation(om[:], pom[:], ACTF.Identity, bias=bom_sb[:], scale=1.0)

        sg = sb.tile([128, 512], F32, tag='sg', name='sg')
        nc.scalar.activation(sg[:], om[:], ACTF.Sigmoid)

        # ---- stage 3: index math (fp32, in-place tile reuse) ----
        def mtile(tag, dt=F32):
            return sbm.tile([128, 512], dt, tag=tag, name=tag)
        P = mtile('P')          # becomes Wf
        nc.vector.tensor_tensor(P[:], om[:], base_sb[:], ALU.add)
        nc.vector.tensor_scalar(P[:], P[:], 96.5, 14.0, ALU.min, ALU.max)
        T32 = mtile('T32', I32)
        nc.vector.tensor_copy(T32[:], P[:])
        Tf = mtile('Tf')        # becomes F (floor)
        nc.vector.tensor_copy(Tf[:], T32[:])
        Gg = mtile('Gg')        # becomes V0
        nc.vector.tensor_tensor(Gg[:], Tf[:], P[:], ALU.is_gt)
        nc.vector.tensor_tensor(Tf[:], Tf[:], Gg[:], ALU.subtract)
        nc.vector.tensor_tensor(P[:], P[:], Tf[:], ALU.subtract)
        Ff, Wf = Tf, P
        Vt = mtile('Vt')
        V0 = Gg
        nc.vector.tensor_scalar(V0[:], Ff[:], 16.0, None, ALU.is_ge)
        nc.vector.tensor_scalar(Vt[:], Ff[:], 80.0, None, ALU.is_lt)
        nc.vector.tensor_tensor(V0[:], V0[:], Vt[:], ALU.mult)
        V1 = mtile('V1')
        nc.vector.tensor_scalar(V1[:], Ff[:], 15.0, None, ALU.is_ge)
        nc.vector.tensor_scalar(Vt[:], Ff[:], 79.0, None, ALU.is_lt)
        nc.vector.tensor_tensor(V1[:], V1[:], Vt[:], ALU.mult)
        W0 = mtile('W0')
        nc.vector.tensor_scalar(W0[:], Wf[:], -1.0, 1.0, ALU.mult, ALU.add)
        nc.vector.tensor_tensor(W0[:], W0[:], V0[:], ALU.mult)
        W1 = Wf
        nc.vector.tensor_tensor(W1[:], Wf[:], V1[:], ALU.mult)
        # single gather coordinate: clamp(floor, 15, 79) serves y and x rows
        AxC = V1
        nc.vector.tensor_scalar(AxC[:], Ff[:], 79.0, 15.0, ALU.min, ALU.max)

        # ---- stage 4: idx assembly -> idxf [9, 2048] f32 (pixel-major) ----
        mc4 = mcomb_sb[:].rearrange('p (q m) -> p q m', q=4)
        idxf = sb.tile([9, 2048], F32, tag='idxf', name='idxf')
        with tc.tile_pool(name='psI', bufs=2, space='PSUM') as psI:
            for q in range(4):
                pidx = psI.tile([9, 512], F32, name='pidx')
                nc.tensor.matmul(pidx[:], mc4[:, q, :], AxC[:], start=True, stop=True)
                nc.scalar.activation(idxf[:, 512 * q:512 * (q + 1)], pidx[:],
                                      ACTF.Identity, bias=bcomb_sb[:], scale=1.0)
        # wrap for dma_gather, grouped so a 3-tap batch has one contiguous
        # idx window: idx16[j, 576*hb + 64*t + 8*bbl + a] = idxf[t, 128*(8*hb+bbl) + 16*a + j]
        idx16 = sb.tile([128, 2 * 9 * 64], I16, tag='idx16', name='idx16')
        tsb = sb.tile([128, 16 * 9], F32, tag='tsb', name='tsb')
        tsb3 = tsb[:].rearrange('p (b t) -> p b t', b=16)
        with tc.tile_pool(name='psT', bufs=2, space='PSUM') as psT:
            for bb in range(16):
                pT = psT.tile([128, 9], F32, name='pT')
                nc.tensor.transpose(pT[:], idxf[:, 128 * bb:128 * (bb + 1)],
                                    identf[0:9, 0:9], tile_position=(0, 0))
                nc.vector.tensor_copy(tsb3[:, bb, :], pT[:])
            for a in range(8):
                pW = psT.tile([16, 144], F32, name='pW')
                nc.tensor.matmul(pW[:], identf[:, 16 * a:16 * (a + 1)], tsb[:],
                                 start=True, stop=True)
                dsta = AP(idx16[:].tensor, idx16[:].offset + a,
                          [[9 * 128, 16], [576, 2], [8, 8], [64, 9]])
                nc.vector.tensor_copy(
                    dsta, pW[:].rearrange('p (h b t) -> p h b t', h=2, b=8))
        nc.sync.dma_start(idx16[16:32, :], idx16[0:16, :])
        nc.sync.dma_start(idx16[32:64, :], idx16[0:32, :])
        nc.sync.dma_start(idx16[64:128, :], idx16[0:64, :])

        # ---- blend weights: mask-select, transpose, products ----
        for Wt in (W0, W1):
            nc.vector.tensor_scalar(Wt[:], Wt[:], rmask_sb[:, 0:1], None, ALU.mult)
            nc.vector.scalar_tensor_tensor(Wt[:], sg[:], rmask_sb[:, 1:2], Wt[:],
                                           ALU.mult, ALU.add)
        wprod = sb.tile([128, 16 * 6 * 9], F32, tag='wprod', name='wprod')
        wp3 = wprod[:].rearrange('p (b s t) -> p b s t', b=16, s=6)
        with tc.tile_pool(name='psW', bufs=2, space='PSUM') as psW:
            for b in range(16):
                q, cc = b // 4, b % 4
                pt = psW.tile([128, 54], F32, name='ptw')
                idq = identf[32 * q:32 * q + 27, 32 * q:32 * q + 27]
                nc.tensor.transpose(pt[:, 0:27], W0[32 * q:32 * q + 27, 128 * cc:128 * (cc + 1)],
                                    idq, tile_position=(32 * q, 0))
                nc.tensor.transpose(pt[:, 27:54], W1[32 * q:32 * q + 27, 128 * cc:128 * (cc + 1)],
                                    idq, tile_position=(32 * q, 0))
                ta = sbt.tile([128, 54], F32, tag='tall', name='tall')
                nc.vector.tensor_copy(ta[:], pt[:])
                r0 = sbt.tile([128, 9], F32, tag='r0', name='r0')
                nc.vector.tensor_tensor(r0[:], ta[:, 0:9], ta[:, 18:27], ALU.mult)
                r1 = sbt.tile([128, 9], F32, tag='r1', name='r1')
                nc.vector.tensor_tensor(r1[:], ta[:, 27:36], ta[:, 45:54], ALU.mult)
                nc.vector.tensor_tensor(wp3[:, b, 0, :], r0[:], ta[:, 9:18], ALU.mult)
                nc.vector.tensor_tensor(wp3[:, b, 1, :], r0[:], ta[:, 36:45], ALU.mult)
                nc.vector.tensor_tensor(wp3[:, b, 2, :], r1[:], ta[:, 9:18], ALU.mult)
                nc.vector.tensor_tensor(wp3[:, b, 3, :], r1[:], ta[:, 36:45], ALU.mult)

        # ---- stages 5-8: per (half, tap) gather -> blend -> transpose ----
        # einsum accumulates in PSUM inside the loop (delayed one tap so PE
        # never waits on the PSUM->SBUF staging copies)
        nc.sync.dma_start(dw[:], dwpack[:])
        gsems = [nc.alloc_semaphore(f'gsem{q}') for q in range(4)]
        gcnt = [0, 0, 0, 0]
        inap = AP(xq[:].tensor, 0, [[512, 4223], [1, 1024]])
        dw4 = dw[:].rearrange('p (k g o c) -> p k g o c', k=9, g=2, o=2)
        with tc.tile_pool(name='psQ', bufs=2, space='PSUM') as psQ, \
             tc.tile_pool(name='psO', bufs=1, space='PSUM') as psO:
            for hb in range(2):
                po = [psO.tile([128, 512], F32, name=f'po{og}{qq}')
                      for og in range(2) for qq in range(2)]

                def einsum_tap(t, stg3):
                    for og in range(2):
                        for qq in range(2):
                            for cg in range(2):
                                nc.tensor.matmul(
                                    po[2 * og + qq][:], dw4[:, t, cg, og, :],
                                    stg3[:, cg, 512 * qq:512 * (qq + 1)],
                                    start=(t == 0 and cg == 0),
                                    stop=(t == 8 and cg == 1))

                prev = None
                gb = None
                gstart = {t: (t, 1) for t in range(9)}
                for t in range(9):
                    if t in gstart:
                        tb, ntap = gstart[t][0], gstart[t][1]
                        gb = gpool2.tile([128, 8 * ntap, 1024], BF16,
                                         tag=f'gq{ntap}', name=f'gq{ntap}')
                        qn = (9 * hb + tb) % 4
                        c0 = 576 * hb + 64 * t
                        nc.gpsimd.dma_gather(
                            out_ap=gb[:], in_ap=inap,
                            idxs_ap=idx16[:, c0: c0 + 64 * ntap],
                            num_idxs=1024 * ntap, num_idxs_reg=1024 * ntap,
                            elem_size=1024, elem_step=512, queue_num=qn,
                            prepare_only=True, sem=gsems[qn])
                        nc.gpsimd.trigger_dma(count=None, queue_num=qn)
                        gcnt[qn] += 1
                        nc.vector.wait_ge(gsems[qn], 16 * gcnt[qn])
                        nc.scalar.wait_ge(gsems[qn], 16 * gcnt[qn])
                        gt0 = t
                    S = spool.tile([128, 8 * 256], BF16, tag='S', name='S')
                    S3 = S[:].rearrange('p (b n) -> p b n', b=8)
                    for blk in range(8):
                        b = 8 * hb + blk
                        gq4 = gb[:]
                        bl = 8 * (t - gt0) + blk
                        m = sba.tile([128, 256], BF16, tag='m', name='m')
                        m2 = sba.tile([128, 256], BF16, tag='m2', name='m2')
                        a1 = sba.tile([128, 256], BF16, tag='a1', name='a1')
                        a2 = sba.tile([128, 256], BF16, tag='a2', name='a2')
                        nc.vector.tensor_scalar_mul(m[:], gq4[:, bl, 0:256],
                                                    wp3[:, b, 0, t:t + 1])
                        nc.scalar.activation(a1[:], gq4[:, bl, 256:512], ACTF.Identity,
                                             scale=wp3[:, b, 2, t:t + 1])
                        nc.vector.tensor_scalar_mul(m2[:], gq4[:, bl, 512:768],
                                                    wp3[:, b, 1, t:t + 1])
                        nc.scalar.activation(a2[:], gq4[:, bl, 768:1024], ACTF.Identity,
                                             scale=wp3[:, b, 3, t:t + 1])
                        nc.vector.tensor_tensor(m[:], m[:], m2[:], ALU.add)
                        nc.vector.tensor_tensor(a1[:], a1[:], a2[:], ALU.add)
                        nc.vector.tensor_tensor(S3[:, blk, :], m[:], a1[:], ALU.add)
                    stg = stpool.tile([128, 2, 1024], BF16, tag='stg', name='stg')
                    for half in range(2):
                        pq = psQ.tile([128, 1024], BF16, name='pq')
                        for j in range(4):
                            blk = 4 * half + j
                            for cg in range(2):
                                nc.tensor.transpose(
                                    pq[:, 256 * j + 128 * cg: 256 * j + 128 * (cg + 1)],
                                    S3[:, blk, 128 * cg:128 * (cg + 1)], ident[:])
                        pq4 = pq[:].rearrange('p (j g c) -> p j g c', j=4, g=2)
                        for cg in range(2):
                            dstp = stg[:, cg, 512 * half:512 * (half + 1)]
                            if half == 0:
                                nc.scalar.activation(
                                    dstp.rearrange('p (j c) -> p j c', j=4),
                                    pq4[:, :, cg, :], ACTF.Identity)
                            else:
                                nc.vector.tensor_copy(
                                    dstp.rearrange('p (j c) -> p j c', j=4),
                                    pq4[:, :, cg, :])
                    if prev is not None:
                        einsum_tap(prev[0], prev[1])
                    prev = (t, stg[:])
                einsum_tap(prev[0], prev[1])
                # ---- bias + store for this half ----
                for og in range(2):
                    for qq in range(2):
                        q = 2 * hb + qq
                        osb = sbt.tile([128, 512], F32, tag='osb', name='osb')
                        nc.scalar.activation(osb[:], po[2 * og + qq][:], ACTF.Identity,
                                             bias=bout_sb[:, og:og + 1], scale=1.0)
                        nc.sync.dma_start(out[128 * og:128 * (og + 1), 512 * q:512 * (q + 1)],
                                          osb[:])

    nc.compile()
    return nc


def prep_core_inputs(inputs, b, h):
    tf = np.ascontiguousarray(np.asarray(inputs['template_feat'][b], dtype=np.float32))
    sf = np.ascontiguousarray(np.asarray(inputs['search_feat'][b], dtype=np.float32))
    offset_w = np.asarray(inputs['offset_w'], dtype=np.float32)
    offset_b = np.asarray(inputs['offset_b'], dtype=np.float32)
    mask_w = np.asarray(inputs['mask_w'], dtype=np.float32)
    mask_b = np.asarray(inputs['mask_b'], dtype=np.float32)
    deform_w = np.asarray(inputs['deform_w'], dtype=np.float32)
    deform_b = np.asarray(inputs['deform_b'], dtype=np.float32)

    tplp = np.zeros((256, 23, 32), np.float32)
    for j in range(19):
        tplp[:, j] = tf[:, min(max(16 * h - 1 + j, 0), 31)]
    if h == 0:
        tplp[:, 21] = tf[:, 15]
        tplp[:, 22] = tf[:, 16]
    else:
        tplp[:, 19] = tf[:, 15]
        tplp[:, 20] = tf[:, 16]

    srch66 = np.zeros((256, 34, 66), np.float32)
    for i in range(34):
        r = 32 * h - 1 + i
        if 0 <= r <= 63:
            srch66[:, i, 1:65] = sf[:, r]

    # quad layout: xq[r] = [pix(r-65), pix(r-1)]; one idx r fetches
    # rows r, r+1 = [TL, BL, TR, BR] corners (2KB).
    sfp = sf.reshape(256, 4096).T  # [4096 px, 256 ch]
    xquad = np.zeros((4224, 512), np.float32)
    xquad[65:65 + 4096, 0:256] = sfp
    xquad[1:1 + 4096, 256:512] = sfp

    wpack = np.zeros((128, 4, 9, 32), np.float32)
    for g in range(4):
        for t, (ky, kx) in enumerate(TAPS):
            cs = slice(128 * g, 128 * (g + 1))
            wpack[:, g, t, 0:9] = offset_w[0::2, cs, ky + 1, kx + 1].T
            wpack[:, g, t, 9:18] = offset_w[1::2, cs, ky + 1, kx + 1].T
            if ky == 0 and kx == 0:
                wpack[:, g, t, 18:27] = mask_w[:, cs, 0, 0].T
    wk = deform_w.reshape(256, 256, 3, 3)
    dwp = np.zeros((128, 9, 2, 2, 128), np.float32)
    for t in range(9):
        ky, kx = TAPS[t]
        for cg in range(2):
            for og in range(2):
                dwp[:, t, cg, og, :] = wk[128 * og:128 * (og + 1),
                                          128 * cg:128 * (cg + 1), ky + 1, kx + 1].T

    basei = np.zeros((128, 512), np.float32)
    col = np.arange(512)
    for q in range(4):
        for m in range(9):
            basei[32 * q + m] = 32 * h + 8 * q + col // 64 + TAPS[m][0] + 16
            basei[32 * q + 9 + m] = col % 64 + TAPS[m][1] + 16

    mcomb = np.zeros((128, 4, 9), np.float32)
    for q in range(4):
        for t in range(9):
            mcomb[32 * q + t, q, t] = 64.0
            mcomb[32 * q + 9 + t, q, t] = 1.0
    bias_comb = np.full((9, 1), float(C0), np.float32)

    bias_om = np.zeros((128, 1), np.float32)
    for q in range(4):
        bias_om[32 * q + 0:32 * q + 9, 0] = offset_b[0::2]
        bias_om[32 * q + 9:32 * q + 18, 0] = offset_b[1::2]
        bias_om[32 * q + 18:32 * q + 27, 0] = mask_b

    rmaski = np.zeros((128, 2), np.float32)
    for q in range(4):
        rmaski[32 * q:32 * q + 18, 0] = 1.0
        rmaski[32 * q + 18:32 * q + 32, 1] = 1.0

    return {
        'rmaski': rmaski,
        'tplp': tplp.reshape(256, 23 * 32).astype(BF16NP),
        'srch66': srch66.reshape(256, 34 * 66).astype(BF16NP),
        'xq': xquad.astype(BF16NP),
        'wpack': wpack.reshape(128, 4 * 9 * 32).astype(BF16NP),
        'dwpack': dwp.reshape(128, 9 * 2 * 2 * 128).astype(BF16NP),
        'basei': basei,
        'mcomb': mcomb.reshape(128, 4 * 9),
        'bias_comb': bias_comb,
        'bias_om': bias_om,
        'bias_out': deform_b.reshape(256, 1).astype(np.float32),
    }


def kernel(**inputs):
    key = 'v2'
    if key not in _NC_CACHE:
        _NC_CACHE[key] = build_nc()
    nc = _NC_CACHE[key]
    in_maps = [prep_core_inputs(inputs, ci // 2, ci % 2) for ci in range(8)]
    res = run_bass_kernel_spmd(nc, in_maps, core_ids=list(range(8)))
    global LAST_RESULT
    LAST_RESULT = res
    out = np.zeros((4, 256, 64, 64), np.float32)
    for ci in range(8):
        b, h = ci // 2, ci % 2
        out[b][:, 32 * h:32 * h + 32, :] = res.results[ci]['out'].reshape(256, 32, 64)
    return out


# revision 34
# speedup vs baseline: 1.0503x; 1.0458x over previous
"""Self-contained Trainium2 kernel for nn_DynamicCrossAttention_40286793236903.

kernel(**inputs) takes the FULL inputs (as produced by setup_inputs) and
returns the FULL [4, 256, 64, 64] float32 output.

Sharding: pure data parallel over (batch, image-half): core ci handles
sample b=ci//2, output rows 32*(ci%2)..32*(ci%2)+31. One SPMD Bass program
runs on all 8 cores; all per-core variation is carried in the input data.

Pipeline per core (all feature data bf16 on device):
  1. upsample template 32x32 -> 64x64 (half-pixel bilinear) and build the
     padded combined tensor [512ch, 34, 66] together with the search half.
  2. offsets+mask 3x3 conv as 36 accumulating PE matmuls per col-quarter.
  3. fp32 index math: sample coords, floor/frac, validity-masked bilinear
     weights; a single clamp serves both the y and x gather coordinate.
  4. gather index assembly (PE transposes) into the 16-partition wrapped
     i16 layout dma_gather wants; one index per (tap, pixel) addresses all
     four bilinear corners through the host-staged quad layout
     xq[r] = [pix(r-65), pix(r-1)]  (2KB per descriptor).
  5. per (half, tap): one SWDGE dma_gather (1024 idxs x 2KB), then a fused
     4-op DVE blend chain per 128-pixel block, PE transpose to channel
     major, and a per-half einsum against the deform weights overlapping
     the other half's loop.
"""
import numpy as np
from contextlib import ExitStack

import ml_dtypes
import concourse.bass as bass
import concourse.mybir as mybir
import concourse.tile as tile
from concourse import bacc
from concourse.bass import AP
from concourse.bass_utils import run_bass_kernel_spmd
from concourse.masks import make_identity

F32 = mybir.dt.float32
BF16 = mybir.dt.bfloat16
I32 = mybir.dt.int32
I16 = mybir.dt.int16
ALU = mybir.AluOpType
ACTF = mybir.ActivationFunctionType
BF16NP = ml_dtypes.bfloat16

TAPS = [(ky, kx) for ky in (-1, 0, 1) for kx in (-1, 0, 1)]
C0 = -(15 * 64) - 16 + 1  # idx = 64*Y + X + C0 -> quad row (see xq layout)

_NC_CACHE = {}
LAST_RESULT = None


def build_nc():
    nc = bacc.Bacc(None, target_bir_lowering=False, num_swdge_queues=4)

    tplp = nc.dram_tensor('tplp', [256, 23 * 32], BF16, kind='ExternalInput')
    srch66 = nc.dram_tensor('srch66', [256, 34 * 66], BF16, kind='ExternalInput')
    xq = nc.dram_tensor('xq', [4224, 512], BF16, kind='ExternalInput')
    wpack = nc.dram_tensor('wpack', [128, 4 * 9 * 32], BF16, kind='ExternalInput')
    dwpack = nc.dram_tensor('dwpack', [128, 9 * 2 * 2 * 128], BF16, kind='ExternalInput')
    basei = nc.dram_tensor('basei', [128, 512], F32, kind='ExternalInput')
    mcomb = nc.dram_tensor('mcomb', [128, 4 * 9], F32, kind='ExternalInput')
    bias_comb = nc.dram_tensor('bias_comb', [9, 1], F32, kind='ExternalInput')
    bias_om = nc.dram_tensor('bias_om', [128, 1], F32, kind='ExternalInput')
    bias_out = nc.dram_tensor('bias_out', [256, 1], F32, kind='ExternalInput')
    rmaski = nc.dram_tensor('rmaski', [128, 2], F32, kind='ExternalInput')
    out = nc.dram_tensor('out', [256, 2048], F32, kind='ExternalOutput')

    with tile.TileContext(nc) as tc, ExitStack() as ctx:
        sb = ctx.enter_context(tc.tile_pool(name='sb', bufs=1))
        sbm = ctx.enter_context(tc.tile_pool(name='sbm', bufs=1))
        sbt = ctx.enter_context(tc.tile_pool(name='sbt', bufs=2))
        sba = ctx.enter_context(tc.tile_pool(name='sba', bufs=3))
        gpool2 = ctx.enter_context(tc.tile_pool(name='gpool2', bufs=5))
        spool = ctx.enter_context(tc.tile_pool(name='spool', bufs=2))
        stpool = ctx.enter_context(tc.tile_pool(name='stpool', bufs=3))

        ident = sb.tile([128, 128], BF16, tag='ident', name='ident')
        make_identity(nc, ident[:])
        identf = sb.tile([128, 128], F32, tag='identf', name='identf')
        make_identity(nc, identf[:])

        wp = sb.tile([128, 4 * 9 * 32], BF16, tag='wp', name='wp')
        dw = sb.tile([128, 9 * 2 * 2 * 128], BF16, tag='dw', name='dw')
        base_sb = sb.tile([128, 512], F32, tag='base', name='base')
        nc.sync.dma_start(base_sb[:], basei[:])
        mcomb_sb = sb.tile([128, 4 * 9], F32, tag='mcomb', name='mcomb')
        nc.sync.dma_start(mcomb_sb[:], mcomb[:])
        bcomb_sb = sb.tile([9, 1], F32, tag='bcomb', name='bcomb')
        nc.sync.dma_start(bcomb_sb[:], bias_comb[:])
        bom_sb = sb.tile([128, 1], F32, tag='bom', name='bom')
        nc.sync.dma_start(bom_sb[:], bias_om[:])
        bout_sb = sb.tile([128, 2], F32, tag='bout', name='bout')
        nc.sync.dma_start(bout_sb[:], bias_out[:].rearrange('(g p) o -> p (g o)', g=2))
        rmask_sb = sb.tile([128, 2], F32, tag='rmask', name='rmask')
        nc.sync.dma_start(rmask_sb[:], rmaski[:])

        # ---- stage 1: upsample template + build combined [512ch, 34, 66] ----
        with tc.tile_pool(name='convsb', bufs=1) as convsb, \
             tc.tile_pool(name='psA', bufs=1, space='PSUM') as psA:
            tps = []
            for cg in range(2):
                tp = convsb.tile([128, 23 * 32], BF16, tag=f'tp{cg}', name=f'tp{cg}')
                nc.sync.dma_start(tp[:], tplp[128 * cg:128 * (cg + 1), :])
                tps.append(tp)
            scbs = []
            for cg in range(2):
                cb = convsb.tile([128, 34 * 66], BF16, tag=f'comb{cg+2}', name=f'comb{cg+2}')
                nc.sync.dma_start(cb[:], srch66[128 * cg:128 * (cg + 1), :])
                scbs.append(cb)
            nc.sync.dma_start(wp[:], wpack[:])
            comb = []
            for cg in range(2):
                tp = tps[cg]
                tp3 = tp[:].rearrange('p (r w) -> p r w', r=23)
                V = convsb.tile([128, 34 * 32], BF16, tag=f'vt{cg}', name=f'vt{cg}')
                V3 = V[:].rearrange('p (r w) -> p r w', r=34)
                tmp = convsb.tile([128, 34 * 32], BF16, tag=f'ut{cg}', name=f'ut{cg}')
                tmp3 = tmp[:].rearrange('p (r w) -> p r w', r=34)
                # vertical: V[i] = wa*tp[j] + wb*tp[j+1] (ts 4x + tt 2x; no stt)
                nc.vector.tensor_scalar_mul(tmp3[:, 0:16, :], tp3[:, 2:18, :], 0.25)
                nc.scalar.activation(V3[:, 2:34:2, :], tp3[:, 1:17, :], ACTF.Identity, scale=0.75)
                nc.vector.tensor_tensor(V3[:, 2:34:2, :], V3[:, 2:34:2, :], tmp3[:, 0:16, :], ALU.add)
                nc.vector.tensor_scalar_mul(tmp3[:, 0:16, :], tp3[:, 1:17, :], 0.75)
                nc.scalar.activation(V3[:, 1:33:2, :], tp3[:, 0:16, :], ACTF.Identity, scale=0.25)
                nc.vector.tensor_tensor(V3[:, 1:33:2, :], V3[:, 1:33:2, :], tmp3[:, 0:16, :], ALU.add)
                nc.vector.tensor_scalar_mul(tmp3[:, 0:1, :], tp3[:, 20:21, :], 0.25)
                nc.scalar.activation(V3[:, 0:1, :], tp3[:, 19:20, :], ACTF.Identity, scale=0.75)
                nc.vector.tensor_tensor(V3[:, 0:1, :], V3[:, 0:1, :], tmp3[:, 0:1, :], ALU.add)
                nc.vector.tensor_scalar_mul(tmp3[:, 0:1, :], tp3[:, 22:23, :], 0.75)
                nc.scalar.activation(V3[:, 33:34, :], tp3[:, 21:22, :], ACTF.Identity, scale=0.25)
                nc.vector.tensor_tensor(V3[:, 33:34, :], V3[:, 33:34, :], tmp3[:, 0:1, :], ALU.add)
                cb = convsb.tile([128, 34 * 66], BF16, tag=f'comb{cg}', name=f'comb{cg}')
                cb3 = cb[:].rearrange('p (r w) -> p r w', r=34)
                nc.vector.memset(cb[:], 0.0)
                h3 = tmp3
                nc.vector.tensor_scalar_mul(h3[:, :, 0:31], V3[:, :, 1:32], 0.75)
                nc.scalar.activation(cb3[:, :, 3:65:2], V3[:, :, 0:31], ACTF.Identity, scale=0.25)
                nc.vector.tensor_tensor(cb3[:, :, 3:65:2], cb3[:, :, 3:65:2], h3[:, :, 0:31], ALU.add)
                nc.vector.tensor_scalar_mul(h3[:, :, 0:31], V3[:, :, 1:32], 0.25)
                nc.scalar.activation(cb3[:, :, 2:64:2], V3[:, :, 0:31], ACTF.Identity, scale=0.75)
                nc.vector.tensor_tensor(cb3[:, :, 2:64:2], cb3[:, :, 2:64:2], h3[:, :, 0:31], ALU.add)
                nc.vector.tensor_copy(cb3[:, :, 1:2], V3[:, :, 0:1])
                nc.vector.tensor_copy(cb3[:, :, 64:65], V3[:, :, 31:32])
                comb.append(cb)
            comb += scbs

            # ---- stage 2: offsets+mask conv, col-tiled quarters ----
            wp4 = wp[:].rearrange('p (g t m) -> p g t m', g=4, t=9)
            pom = psA.tile([128, 512], F32, name='pom')
            for q in range(4):
                first = True
                for gi, g in enumerate((2, 3, 0, 1)):
                    cb3 = comb[g][:].rearrange('p (r w) -> p r w', r=34)
                    for t, (ky, kx) in enumerate(TAPS):
                        rhs = cb3[:, 8 * q + 1 + ky: 8 * q + 9 + ky, 1 + kx: 65 + kx]
                        nc.tensor.matmul(
                            pom[32 * q:32 * q + 32, :], wp4[:, g, t, :], rhs,
                            start=first, stop=(gi == 3 and t == 8),
                            tile_position=(0, 32 * q))
                        first = False
            om = sb.tile([128, 512], F32, tag='om', name='om')
            nc.scalar.activation(om[:], pom[:], ACTF.Identity, bias=bom_sb[:], scale=1.0)

        sg = sb.tile([128, 512], F32, tag='sg', name='sg')
        nc.scalar.activation(sg[:], om[:], ACTF.Sigmoid)

        # ---- stage 3: index math (fp32, in-place tile reuse) ----
        def mtile(tag, dt=F32):
            return sbm.tile([128, 512], dt, tag=tag, name=tag)
        P = mtile('P')          # becomes Wf
        nc.vector.tensor_tensor(P[:], om[:], base_sb[:], ALU.add)
        nc.vector.tensor_scalar(P[:], P[:], 96.5, 14.0, ALU.min, ALU.max)
        T32 = mtile('T32', I32)
        nc.vector.tensor_copy(T32[:], P[:])
        Tf = mtile('Tf')        # becomes F (floor)
        nc.vector.tensor_copy(Tf[:], T32[:])
        Gg = mtile('Gg')        # becomes V0
        nc.vector.tensor_tensor(Gg[:], Tf[:], P[:], ALU.is_gt)
        nc.vector.tensor_tensor(Tf[:], Tf[:], Gg[:], ALU.subtract)
        nc.vector.tensor_tensor(P[:], P[:], Tf[:], ALU.subtract)
        Ff, Wf = Tf, P
        Vt = mtile('Vt')
        V0 = Gg
        nc.vector.tensor_scalar(V0[:], Ff[:], 16.0, None, ALU.is_ge)
        nc.vector.tensor_scalar(Vt[:], Ff[:], 80.0, None, ALU.is_lt)
        nc.vector.tensor_tensor(V0[:], V0[:], Vt[:], ALU.mult)
        V1 = mtile('V1')
        nc.vector.tensor_scalar(V1[:], Ff[:], 15.0, None, ALU.is_ge)
        nc.vector.tensor_scalar(Vt[:], Ff[:], 79.0, None, ALU.is_lt)
        nc.vector.tensor_tensor(V1[:], V1[:], Vt[:], ALU.mult)
        W0 = mtile('W0')
        nc.vector.tensor_scalar(W0[:], Wf[:], -1.0, 1.0, ALU.mult, ALU.add)
        nc.vector.tensor_tensor(W0[:], W0[:], V0[:], ALU.mult)
        W1 = Wf
        nc.vector.tensor_tensor(W1[:], Wf[:], V1[:], ALU.mult)
        # single gather coordinate: clamp(floor, 15, 79) serves y and x rows
        AxC = V1
        nc.vector.tensor_scalar(AxC[:], Ff[:], 79.0, 15.0, ALU.min, ALU.max)

        # ---- stage 4: idx assembly -> idxf [9, 2048] f32 (pixel-major) ----
        mc4 = mcomb_sb[:].rearrange('p (q m) -> p q m', q=4)
        idxf = sb.tile([9, 2048], F32, tag='idxf', name='idxf')
        with tc.tile_pool(name='psI', bufs=2, space='PSUM') as psI:
            for q in range(4):
                pidx = psI.tile([9, 512], F32, name='pidx')
                nc.tensor.matmul(pidx[:], mc4[:, q, :], AxC[:], start=True, stop=True)
                nc.scalar.activation(idxf[:, 512 * q:512 * (q + 1)], pidx[:],
                                      ACTF.Identity, bias=bcomb_sb[:], scale=1.0)
        # wrap for dma_gather, grouped so a 3-tap batch has one contiguous
        # idx window: idx16[j, 576*hb + 64*t + 8*bbl + a] = idxf[t, 128*(8*hb+bbl) + 16*a + j]
        idx16 = sb.tile([128, 2 * 9 * 64], I16, tag='idx16', name='idx16')
        tsb = sb.tile([128, 16 * 9], F32, tag='tsb', name='tsb')
        tsb3 = tsb[:].rearrange('p (b t) -> p b t', b=16)
        with tc.tile_pool(name='psT', bufs=2, space='PSUM') as psT:
            for bb in range(16):
                pT = psT.tile([128, 9], F32, name='pT')
                nc.tensor.transpose(pT[:], idxf[:, 128 * bb:128 * (bb + 1)],
                                    identf[0:9, 0:9], tile_position=(0, 0))
                nc.vector.tensor_copy(tsb3[:, bb, :], pT[:])
            for a in range(8):
                pW = psT.tile([16, 144], F32, name='pW')
                nc.tensor.matmul(pW[:], identf[:, 16 * a:16 * (a + 1)], tsb[:],
                                 start=True, stop=True)
                dsta = AP(idx16[:].tensor, idx16[:].offset + a,
                          [[9 * 128, 16], [576, 2], [8, 8], [64, 9]])
                nc.vector.tensor_copy(
                    dsta, pW[:].rearrange('p (h b t) -> p h b t', h=2, b=8))
        nc.sync.dma_start(idx16[16:32, :], idx16[0:16, :])
        nc.sync.dma_start(idx16[32:64, :], idx16[0:32, :])
        nc.sync.dma_start(idx16[64:128, :], idx16[0:64, :])

        # ---- blend weights: mask-select, transpose, products ----
        for Wt in (W0, W1):
            nc.vector.tensor_scalar(Wt[:], Wt[:], rmask_sb[:, 0:1], None, ALU.mult)
            nc.vector.scalar_tensor_tensor(Wt[:], sg[:], rmask_sb[:, 1:2], Wt[:],
                                           ALU.mult, ALU.add)
        wprod = sb.tile([128, 16 * 6 * 9], F32, tag='wprod', name='wprod')
        wp3 = wprod[:].rearrange('p (b s t) -> p b s t', b=16, s=6)
        with tc.tile_pool(name='psW', bufs=2, space='PSUM') as psW:
            for b in range(16):
                q, cc = b // 4, b % 4
                pt = psW.tile([128, 54], F32, name='ptw')
                idq = identf[32 * q:32 * q + 27, 32 * q:32 * q + 27]
                nc.tensor.transpose(pt[:, 0:27], W0[32 * q:32 * q + 27, 128 * cc:128 * (cc + 1)],
                                    idq, tile_position=(32 * q, 0))
                nc.tensor.transpose(pt[:, 27:54], W1[32 * q:32 * q + 27, 128 * cc:128 * (cc + 1)],
                                    idq, tile_position=(32 * q, 0))
                ta = sbt.tile([128, 54], F32, tag='tall', name='tall')
                nc.vector.tensor_copy(ta[:], pt[:])
                r0 = sbt.tile([128, 9], F32, tag='r0', name='r0')
                nc.vector.tensor_tensor(r0[:], ta[:, 0:9], ta[:, 18:27], ALU.mult)
                r1 = sbt.tile([128, 9], F32, tag='r1', name='r1')
                nc.vector.tensor_tensor(r1[:], ta[:, 27:36], ta[:, 45:54], ALU.mult)
                nc.vector.tensor_tensor(wp3[:, b, 0, :], r0[:], ta[:, 9:18], ALU.mult)
                nc.vector.tensor_tensor(wp3[:, b, 1, :], r0[:], ta[:, 36:45], ALU.mult)
                nc.vector.tensor_tensor(wp3[:, b, 2, :], r1[:], ta[:, 9:18], ALU.mult)
                nc.vector.tensor_tensor(wp3[:, b, 3, :], r1[:], ta[:, 36:45], ALU.mult)

        # ---- stages 5-8: per (half, tap) gather -> blend -> transpose ----
        # einsum accumulates in PSUM inside the loop (delayed one tap so PE
        # never waits on the PSUM->SBUF staging copies)
        nc.sync.dma_start(dw[:], dwpack[:])
        inap = AP(xq[:].tensor, 0, [[512, 4223], [1, 1024]])
        dw4 = dw[:].rearrange('p (k g o c) -> p k g o c', k=9, g=2, o=2)
        with tc.tile_pool(name='psQ', bufs=2, space='PSUM') as psQ, \
             tc.tile_pool(name='psO', bufs=1, space='PSUM') as psO:
            for hb in range(2):
                po = [psO.tile([128, 512], F32, name=f'po{og}{qq}')
                      for og in range(2) for qq in range(2)]

                def einsum_tap(t, stg3):
                    for og in range(2):
                        for qq in range(2):
                            for cg in range(2):
                                nc.tensor.matmul(
                                    po[2 * og + qq][:], dw4[:, t, cg, og, :],
                                    stg3[:, cg, 512 * qq:512 * (qq + 1)],
                                    start=(t == 0 and cg == 0),
                                    stop=(t == 8 and cg == 1))

                prev = None
                gb = None
                gstart = {t: (t, 1) for t in range(9)}
                for t in range(9):
                    if t in gstart:
                        tb, ntap = gstart[t][0], gstart[t][1]
                        gb = gpool2.tile([128, 8 * ntap, 1024], BF16,
                                         tag=f'gq{ntap}', name=f'gq{ntap}')
                        qn = (9 * hb + tb) % 4
                        c0 = 576 * hb + 64 * t
                        nc.gpsimd.dma_gather(
                            out_ap=gb[:], in_ap=inap,
                            idxs_ap=idx16[:, c0: c0 + 64 * ntap],
                            num_idxs=1024 * ntap, num_idxs_reg=1024 * ntap,
                            elem_size=1024, elem_step=512, queue_num=qn)
                        gt0 = t
                    S = spool.tile([128, 8 * 256], BF16, tag='S', name='S')
                    S3 = S[:].rearrange('p (b n) -> p b n', b=8)
                    for blk in range(8):
                        b = 8 * hb + blk
                        gq4 = gb[:]
                        bl = 8 * (t - gt0) + blk
                        m = sba.tile([128, 256], BF16, tag='m', name='m')
                        m2 = sba.tile([128, 256], BF16, tag='m2', name='m2')
                        a1 = sba.tile([128, 256], BF16, tag='a1', name='a1')
                        a2 = sba.tile([128, 256], BF16, tag='a2', name='a2')
                        nc.vector.tensor_scalar_mul(m[:], gq4[:, bl, 0:256],
                                                    wp3[:, b, 0, t:t + 1])
                        nc.scalar.activation(a1[:], gq4[:, bl, 256:512], ACTF.Identity,
                                             scale=wp3[:, b, 2, t:t + 1])
                        nc.vector.tensor_scalar_mul(m2[:], gq4[:, bl, 512:768],
                                                    wp3[:, b, 1, t:t + 1])
                        nc.scalar.activation(a2[:], gq4[:, bl, 768:1024], ACTF.Identity,
                                             scale=wp3[:, b, 3, t:t + 1])
                        nc.vector.tensor_tensor(m[:], m[:], m2[:], ALU.add)
                        nc.vector.tensor_tensor(a1[:], a1[:], a2[:], ALU.add)
                        nc.vector.tensor_tensor(S3[:, blk, :], m[:], a1[:], ALU.add)
                    stg = stpool.tile([128, 2, 1024], BF16, tag='stg', name='stg')
                    for half in range(2):
                        pq = psQ.tile([128, 1024], BF16, name='pq')
                        for j in range(4):
                            blk = 4 * half + j
                            for cg in range(2):
                                nc.tensor.transpose(
                                    pq[:, 256 * j + 128 * cg: 256 * j + 128 * (cg + 1)],
                                    S3[:, blk, 128 * cg:128 * (cg + 1)], ident[:])
                        pq4 = pq[:].rearrange('p (j g c) -> p j g c', j=4, g=2)
                        for cg in range(2):
                            dstp = stg[:, cg, 512 * half:512 * (half + 1)]
                            if half == 0:
                                nc.scalar.activation(
                                    dstp.rearrange('p (j c) -> p j c', j=4),
                                    pq4[:, :, cg, :], ACTF.Identity)
                            else:
                                nc.vector.tensor_copy(
                                    dstp.rearrange('p (j c) -> p j c', j=4),
                                    pq4[:, :, cg, :])
                    if prev is not None:
                        einsum_tap(prev[0], prev[1])
                    prev = (t, stg[:])
                einsum_tap(prev[0], prev[1])
                # ---- bias + store for this half ----
                for og in range(2):
                    for qq in range(2):
                        q = 2 * hb + qq
                        osb = sbt.tile([128, 512], F32, tag='osb', name='osb')
                        nc.scalar.activation(osb[:], po[2 * og + qq][:], ACTF.Identity,
                                             bias=bout_sb[:, og:og + 1], scale=1.0)
                        nc.sync.dma_start(out[128 * og:128 * (og + 1), 512 * q:512 * (q + 1)],
                                          osb[:])

    nc.compile()
    return nc


def prep_core_inputs(inputs, b, h):
    tf = np.ascontiguousarray(np.asarray(inputs['template_feat'][b], dtype=np.float32))
    sf = np.ascontiguousarray(np.asarray(inputs['search_feat'][b], dtype=np.float32))
    offset_w = np.asarray(inputs['offset_w'], dtype=np.float32)
    offset_b = np.asarray(inputs['offset_b'], dtype=np.float32)
    mask_w = np.asarray(inputs['mask_w'], dtype=np.float32)
    mask_b = np.asarray(inputs['mask_b'], dtype=np.float32)
    deform_w = np.asarray(inputs['deform_w'], dtype=np.float32)
    deform_b = np.asarray(inputs['deform_b'], dtype=np.float32)

    tplp = np.zeros((256, 23, 32), np.float32)
    for j in range(19):
        tplp[:, j] = tf[:, min(max(16 * h - 1 + j, 0), 31)]
    if h == 0:
        tplp[:, 21] = tf[:, 15]
        tplp[:, 22] = tf[:, 16]
    else:
        tplp[:, 19] = tf[:, 15]
        tplp[:, 20] = tf[:, 16]

    srch66 = np.zeros((256, 34, 66), np.float32)
    for i in range(34):
        r = 32 * h - 1 + i
        if 0 <= r <= 63:
            srch66[:, i, 1:65] = sf[:, r]

    # quad layout: xq[r] = [pix(r-65), pix(r-1)]; one idx r fetches
    # rows r, r+1 = [TL, BL, TR, BR] corners (2KB).
    sfp = sf.reshape(256, 4096).T  # [4096 px, 256 ch]
    xquad = np.zeros((4224, 512), np.float32)
    xquad[65:65 + 4096, 0:256] = sfp
    xquad[1:1 + 4096, 256:512] = sfp

    wpack = np.zeros((128, 4, 9, 32), np.float32)
    for g in range(4):
        for t, (ky, kx) in enumerate(TAPS):
            cs = slice(128 * g, 128 * (g + 1))
            wpack[:, g, t, 0:9] = offset_w[0::2, cs, ky + 1, kx + 1].T
            wpack[:, g, t, 9:18] = offset_w[1::2, cs, ky + 1, kx + 1].T
            if ky == 0 and kx == 0:
                wpack[:, g, t, 18:27] = mask_w[:, cs, 0, 0].T
    wk = deform_w.reshape(256, 256, 3, 3)
    dwp = np.zeros((128, 9, 2, 2, 128), np.float32)
    for t in range(9):
        ky, kx = TAPS[t]
        for cg in range(2):
            for og in range(2):
                dwp[:, t, cg, og, :] = wk[128 * og:128 * (og + 1),
                                          128 * cg:128 * (cg + 1), ky + 1, kx + 1].T

    basei = np.zeros((128, 512), np.float32)
    col = np.arange(512)
    for q in range(4):
        for m in range(9):
            basei[32 * q + m] = 32 * h + 8 * q + col // 64 + TAPS[m][0] + 16
            basei[32 * q + 9 + m] = col % 64 + TAPS[m][1] + 16

    mcomb = np.zeros((128, 4, 9), np.float32)
    for q in range(4):
        for t in range(9):
            mcomb[32 * q + t, q, t] = 64.0
            mcomb[32 * q + 9 + t, q, t] = 1.0
    bias_comb = np.full((9, 1), float(C0), np.float32)

    bias_om = np.zeros((128, 1), np.float32)
    for q in range(4):
        bias_om[32 * q + 0:32 * q + 9, 0] = offset_b[0::2]
        bias_om[32 * q + 9:32 * q + 18, 0] = offset_b[1::2]
        bias_om[32 * q + 18:32 * q + 27, 0] = mask_b

    rmaski = np.zeros((128, 2), np.float32)
    for q in range(4):
        rmaski[32 * q:32 * q + 18, 0] = 1.0
        rmaski[32 * q + 18:32 * q + 32, 1] = 1.0

    return {
        'rmaski': rmaski,
        'tplp': tplp.reshape(256, 23 * 32).astype(BF16NP),
        'srch66': srch66.reshape(256, 34 * 66).astype(BF16NP),
        'xq': xquad.astype(BF16NP),
        'wpack': wpack.reshape(128, 4 * 9 * 32).astype(BF16NP),
        'dwpack': dwp.reshape(128, 9 * 2 * 2 * 128).astype(BF16NP),
        'basei': basei,
        'mcomb': mcomb.reshape(128, 4 * 9),
        'bias_comb': bias_comb,
        'bias_om': bias_om,
        'bias_out': deform_b.reshape(256, 1).astype(np.float32),
    }


def kernel(**inputs):
    key = 'v2'
    if key not in _NC_CACHE:
        _NC_CACHE[key] = build_nc()
    nc = _NC_CACHE[key]
    in_maps = [prep_core_inputs(inputs, ci // 2, ci % 2) for ci in range(8)]
    res = run_bass_kernel_spmd(nc, in_maps, core_ids=list(range(8)))
    global LAST_RESULT
    LAST_RESULT = res
    out = np.zeros((4, 256, 64, 64), np.float32)
    for ci in range(8):
        b, h = ci // 2, ci % 2
        out[b][:, 32 * h:32 * h + 32, :] = res.results[ci]['out'].reshape(256, 32, 64)
    return out


# revision 35
# speedup vs baseline: 1.0667x; 1.0156x over previous
"""Self-contained Trainium2 kernel for nn_DynamicCrossAttention_40286793236903.

kernel(**inputs) takes the FULL inputs (as produced by setup_inputs) and
returns the FULL [4, 256, 64, 64] float32 output.

Sharding: pure data parallel over (batch, image-half): core ci handles
sample b=ci//2, output rows 32*(ci%2)..32*(ci%2)+31. One SPMD Bass program
runs on all 8 cores; all per-core variation is carried in the input data.

Pipeline per core (all feature data bf16 on device):
  1. upsample template 32x32 -> 64x64 (half-pixel bilinear) and build the
     padded combined tensor [512ch, 34, 66] together with the search half.
  2. offsets+mask 3x3 conv as 36 accumulating PE matmuls per col-quarter.
  3. fp32 index math: sample coords, floor/frac, validity-masked bilinear
     weights; a single clamp serves both the y and x gather coordinate.
  4. gather index assembly (PE transposes) into the 16-partition wrapped
     i16 layout dma_gather wants; one index per (tap, pixel) addresses all
     four bilinear corners through the host-staged quad layout
     xq[r] = [pix(r-65), pix(r-1)]  (2KB per descriptor).
  5. per (half, tap): one SWDGE dma_gather (1024 idxs x 2KB), then a fused
     4-op DVE blend chain per 128-pixel block, PE transpose to channel
     major, and a per-half einsum against the deform weights overlapping
     the other half's loop.
"""
import numpy as np
from contextlib import ExitStack

import ml_dtypes
import concourse.bass as bass
import concourse.mybir as mybir
import concourse.tile as tile
from concourse import bacc
from concourse.bass import AP
from concourse.bass_utils import run_bass_kernel_spmd
from concourse.masks import make_identity

F32 = mybir.dt.float32
BF16 = mybir.dt.bfloat16
I32 = mybir.dt.int32
I16 = mybir.dt.int16
ALU = mybir.AluOpType
ACTF = mybir.ActivationFunctionType
BF16NP = ml_dtypes.bfloat16

TAPS = [(ky, kx) for ky in (-1, 0, 1) for kx in (-1, 0, 1)]
C0 = -(15 * 64) - 16 + 1  # idx = 64*Y + X + C0 -> quad row (see xq layout)

_NC_CACHE = {}
LAST_RESULT = None


def build_nc():
    nc = bacc.Bacc(None, target_bir_lowering=False, num_swdge_queues=4)

    tplp = nc.dram_tensor('tplp', [256, 23 * 32], BF16, kind='ExternalInput')
    srch66 = nc.dram_tensor('srch66', [256, 34 * 66], BF16, kind='ExternalInput')
    xq = nc.dram_tensor('xq', [4224, 512], BF16, kind='ExternalInput')
    wpack = nc.dram_tensor('wpack', [128, 4 * 9 * 32], BF16, kind='ExternalInput')
    dwpack = nc.dram_tensor('dwpack', [128, 9 * 2 * 2 * 128], BF16, kind='ExternalInput')
    basei = nc.dram_tensor('basei', [128, 512], F32, kind='ExternalInput')
    mcomb = nc.dram_tensor('mcomb', [128, 4 * 9], F32, kind='ExternalInput')
    bias_comb = nc.dram_tensor('bias_comb', [9, 1], F32, kind='ExternalInput')
    bias_om = nc.dram_tensor('bias_om', [128, 1], F32, kind='ExternalInput')
    bias_out = nc.dram_tensor('bias_out', [256, 1], F32, kind='ExternalInput')
    rmaski = nc.dram_tensor('rmaski', [128, 2], F32, kind='ExternalInput')
    out = nc.dram_tensor('out', [256, 2048], F32, kind='ExternalOutput')

    with tile.TileContext(nc) as tc, ExitStack() as ctx:
        sb = ctx.enter_context(tc.tile_pool(name='sb', bufs=1))
        sbm = ctx.enter_context(tc.tile_pool(name='sbm', bufs=1))
        sbt = ctx.enter_context(tc.tile_pool(name='sbt', bufs=2))
        sba = ctx.enter_context(tc.tile_pool(name='sba', bufs=3))
        gpool2 = ctx.enter_context(tc.tile_pool(name='gpool2', bufs=6))
        spool = ctx.enter_context(tc.tile_pool(name='spool', bufs=2))
        stpool = ctx.enter_context(tc.tile_pool(name='stpool', bufs=3))

        ident = sb.tile([128, 128], BF16, tag='ident', name='ident')
        make_identity(nc, ident[:])
        identf = sb.tile([128, 128], F32, tag='identf', name='identf')
        make_identity(nc, identf[:])

        wp = sb.tile([128, 4 * 9 * 32], BF16, tag='wp', name='wp')
        dw = sb.tile([128, 9 * 2 * 2 * 128], BF16, tag='dw', name='dw')
        base_sb = sb.tile([128, 512], F32, tag='base', name='base')
        nc.sync.dma_start(base_sb[:], basei[:])
        mcomb_sb = sb.tile([128, 4 * 9], F32, tag='mcomb', name='mcomb')
        nc.sync.dma_start(mcomb_sb[:], mcomb[:])
        bcomb_sb = sb.tile([9, 1], F32, tag='bcomb', name='bcomb')
        nc.sync.dma_start(bcomb_sb[:], bias_comb[:])
        bom_sb = sb.tile([128, 1], F32, tag='bom', name='bom')
        nc.sync.dma_start(bom_sb[:], bias_om[:])
        bout_sb = sb.tile([128, 2], F32, tag='bout', name='bout')
        nc.sync.dma_start(bout_sb[:], bias_out[:].rearrange('(g p) o -> p (g o)', g=2))
        rmask_sb = sb.tile([128, 2], F32, tag='rmask', name='rmask')
        nc.sync.dma_start(rmask_sb[:], rmaski[:])

        # ---- stage 1: upsample template + build combined [512ch, 34, 66] ----
        with tc.tile_pool(name='convsb', bufs=1) as convsb, \
             tc.tile_pool(name='psA', bufs=1, space='PSUM') as psA:
            tps = []
            for cg in range(2):
                tp = convsb.tile([128, 23 * 32], BF16, tag=f'tp{cg}', name=f'tp{cg}')
                nc.sync.dma_start(tp[:], tplp[128 * cg:128 * (cg + 1), :])
                tps.append(tp)
            scbs = []
            for cg in range(2):
                cb = convsb.tile([128, 34 * 66], BF16, tag=f'comb{cg+2}', name=f'comb{cg+2}')
                nc.sync.dma_start(cb[:], srch66[128 * cg:128 * (cg + 1), :])
                scbs.append(cb)
            nc.sync.dma_start(wp[:], wpack[:])
            comb = []
            for cg in range(2):
                tp = tps[cg]
                tp3 = tp[:].rearrange('p (r w) -> p r w', r=23)
                V = convsb.tile([128, 34 * 32], BF16, tag=f'vt{cg}', name=f'vt{cg}')
                V3 = V[:].rearrange('p (r w) -> p r w', r=34)
                tmp = convsb.tile([128, 34 * 32], BF16, tag=f'ut{cg}', name=f'ut{cg}')
                tmp3 = tmp[:].rearrange('p (r w) -> p r w', r=34)
                # vertical: V[i] = wa*tp[j] + wb*tp[j+1] (ts 4x + tt 2x; no stt)
                nc.vector.tensor_scalar_mul(tmp3[:, 0:16, :], tp3[:, 2:18, :], 0.25)
                nc.scalar.activation(V3[:, 2:34:2, :], tp3[:, 1:17, :], ACTF.Identity, scale=0.75)
                nc.vector.tensor_tensor(V3[:, 2:34:2, :], V3[:, 2:34:2, :], tmp3[:, 0:16, :], ALU.add)
                nc.vector.tensor_scalar_mul(tmp3[:, 0:16, :], tp3[:, 1:17, :], 0.75)
                nc.scalar.activation(V3[:, 1:33:2, :], tp3[:, 0:16, :], ACTF.Identity, scale=0.25)
                nc.vector.tensor_tensor(V3[:, 1:33:2, :], V3[:, 1:33:2, :], tmp3[:, 0:16, :], ALU.add)
                nc.vector.tensor_scalar_mul(tmp3[:, 0:1, :], tp3[:, 20:21, :], 0.25)
                nc.scalar.activation(V3[:, 0:1, :], tp3[:, 19:20, :], ACTF.Identity, scale=0.75)
                nc.vector.tensor_tensor(V3[:, 0:1, :], V3[:, 0:1, :], tmp3[:, 0:1, :], ALU.add)
                nc.vector.tensor_scalar_mul(tmp3[:, 0:1, :], tp3[:, 22:23, :], 0.75)
                nc.scalar.activation(V3[:, 33:34, :], tp3[:, 21:22, :], ACTF.Identity, scale=0.25)
                nc.vector.tensor_tensor(V3[:, 33:34, :], V3[:, 33:34, :], tmp3[:, 0:1, :], ALU.add)
                cb = convsb.tile([128, 34 * 66], BF16, tag=f'comb{cg}', name=f'comb{cg}')
                cb3 = cb[:].rearrange('p (r w) -> p r w', r=34)
                nc.vector.memset(cb[:], 0.0)
                h3 = tmp3
                nc.vector.tensor_scalar_mul(h3[:, :, 0:31], V3[:, :, 1:32], 0.75)
                nc.scalar.activation(cb3[:, :, 3:65:2], V3[:, :, 0:31], ACTF.Identity, scale=0.25)
                nc.vector.tensor_tensor(cb3[:, :, 3:65:2], cb3[:, :, 3:65:2], h3[:, :, 0:31], ALU.add)
                nc.vector.tensor_scalar_mul(h3[:, :, 0:31], V3[:, :, 1:32], 0.25)
                nc.scalar.activation(cb3[:, :, 2:64:2], V3[:, :, 0:31], ACTF.Identity, scale=0.75)
                nc.vector.tensor_tensor(cb3[:, :, 2:64:2], cb3[:, :, 2:64:2], h3[:, :, 0:31], ALU.add)
                nc.vector.tensor_copy(cb3[:, :, 1:2], V3[:, :, 0:1])
                nc.vector.tensor_copy(cb3[:, :, 64:65], V3[:, :, 31:32])
                comb.append(cb)
            comb += scbs

            # ---- stage 2: offsets+mask conv, col-tiled quarters ----
            wp4 = wp[:].rearrange('p (g t m) -> p g t m', g=4, t=9)
            pom = psA.tile([128, 512], F32, name='pom')
            for q in range(4):
                first = True
                for gi, g in enumerate((2, 3, 0, 1)):
                    cb3 = comb[g][:].rearrange('p (r w) -> p r w', r=34)
                    for t, (ky, kx) in enumerate(TAPS):
                        rhs = cb3[:, 8 * q + 1 + ky: 8 * q + 9 + ky, 1 + kx: 65 + kx]
                        nc.tensor.matmul(
                            pom[32 * q:32 * q + 32, :], wp4[:, g, t, :], rhs,
                            start=first, stop=(gi == 3 and t == 8),
                            tile_position=(0, 32 * q))
                        first = False
            om = sb.tile([128, 512], F32, tag='om', name='om')
            nc.scalar.activation(om[:], pom[:], ACTF.Identity, bias=bom_sb[:], scale=1.0)

        sg = sb.tile([128, 512], F32, tag='sg', name='sg')
        nc.scalar.activation(sg[:], om[:], ACTF.Sigmoid)

        # ---- stage 3: index math (fp32, in-place tile reuse) ----
        def mtile(tag, dt=F32):
            return sbm.tile([128, 512], dt, tag=tag, name=tag)
        P = mtile('P')          # becomes Wf
        nc.vector.tensor_tensor(P[:], om[:], base_sb[:], ALU.add)
        nc.vector.tensor_scalar(P[:], P[:], 96.5, 14.0, ALU.min, ALU.max)
        T32 = mtile('T32', I32)
        nc.vector.tensor_copy(T32[:], P[:])
        Tf = mtile('Tf')        # becomes F (floor)
        nc.vector.tensor_copy(Tf[:], T32[:])
        Gg = mtile('Gg')        # becomes V0
        nc.vector.tensor_tensor(Gg[:], Tf[:], P[:], ALU.is_gt)
        nc.vector.tensor_tensor(Tf[:], Tf[:], Gg[:], ALU.subtract)
        nc.vector.tensor_tensor(P[:], P[:], Tf[:], ALU.subtract)
        Ff, Wf = Tf, P
        Vt = mtile('Vt')
        V0 = Gg
        nc.vector.tensor_scalar(V0[:], Ff[:], 16.0, None, ALU.is_ge)
        nc.vector.tensor_scalar(Vt[:], Ff[:], 80.0, None, ALU.is_lt)
        nc.vector.tensor_tensor(V0[:], V0[:], Vt[:], ALU.mult)
        V1 = mtile('V1')
        nc.vector.tensor_scalar(V1[:], Ff[:], 15.0, None, ALU.is_ge)
        nc.vector.tensor_scalar(Vt[:], Ff[:], 79.0, None, ALU.is_lt)
        nc.vector.tensor_tensor(V1[:], V1[:], Vt[:], ALU.mult)
        W0 = mtile('W0')
        nc.vector.tensor_scalar(W0[:], Wf[:], -1.0, 1.0, ALU.mult, ALU.add)
        nc.vector.tensor_tensor(W0[:], W0[:], V0[:], ALU.mult)
        W1 = Wf
        nc.vector.tensor_tensor(W1[:], Wf[:], V1[:], ALU.mult)
        # single gather coordinate: clamp(floor, 15, 79) serves y and x rows
        AxC = V1
        nc.vector.tensor_scalar(AxC[:], Ff[:], 79.0, 15.0, ALU.min, ALU.max)

        # ---- stage 4: idx assembly -> idxf [9, 2048] f32 (pixel-major) ----
        mc4 = mcomb_sb[:].rearrange('p (q m) -> p q m', q=4)
        idxf = sb.tile([9, 2048], F32, tag='idxf', name='idxf')
        with tc.tile_pool(name='psI', bufs=2, space='PSUM') as psI:
            for q in range(4):
                pidx = psI.tile([9, 512], F32, name='pidx')
                nc.tensor.matmul(pidx[:], mc4[:, q, :], AxC[:], start=True, stop=True)
                nc.scalar.activation(idxf[:, 512 * q:512 * (q + 1)], pidx[:],
                                      ACTF.Identity, bias=bcomb_sb[:], scale=1.0)
        # wrap for dma_gather, grouped so a 3-tap batch has one contiguous
        # idx window: idx16[j, 576*hb + 64*t + 8*bbl + a] = idxf[t, 128*(8*hb+bbl) + 16*a + j]
        idx16 = sb.tile([128, 2 * 9 * 64], I16, tag='idx16', name='idx16')
        tsb = sb.tile([128, 16 * 9], F32, tag='tsb', name='tsb')
        tsb3 = tsb[:].rearrange('p (b t) -> p b t', b=16)
        with tc.tile_pool(name='psT', bufs=2, space='PSUM') as psT:
            for bb in range(16):
                pT = psT.tile([128, 9], F32, name='pT')
                nc.tensor.transpose(pT[:], idxf[:, 128 * bb:128 * (bb + 1)],
                                    identf[0:9, 0:9], tile_position=(0, 0))
                nc.vector.tensor_copy(tsb3[:, bb, :], pT[:])
            for a in range(8):
                pW = psT.tile([16, 144], F32, name='pW')
                nc.tensor.matmul(pW[:], identf[:, 16 * a:16 * (a + 1)], tsb[:],
                                 start=True, stop=True)
                dsta = AP(idx16[:].tensor, idx16[:].offset + a,
                          [[9 * 128, 16], [576, 2], [8, 8], [64, 9]])
                nc.vector.tensor_copy(
                    dsta, pW[:].rearrange('p (h b t) -> p h b t', h=2, b=8))
        nc.sync.dma_start(idx16[16:32, :], idx16[0:16, :])
        nc.sync.dma_start(idx16[32:64, :], idx16[0:32, :])
        nc.sync.dma_start(idx16[64:128, :], idx16[0:64, :])

        # ---- blend weights: mask-select, transpose, products ----
        for Wt in (W0, W1):
            nc.vector.tensor_scalar(Wt[:], Wt[:], rmask_sb[:, 0:1], None, ALU.mult)
            nc.vector.scalar_tensor_tensor(Wt[:], sg[:], rmask_sb[:, 1:2], Wt[:],
                                           ALU.mult, ALU.add)
        wprod = sb.tile([128, 16 * 6 * 9], F32, tag='wprod', name='wprod')
        wp3 = wprod[:].rearrange('p (b s t) -> p b s t', b=16, s=6)
        with tc.tile_pool(name='psW', bufs=2, space='PSUM') as psW:
            for b in range(16):
                q, cc = b // 4, b % 4
                pt = psW.tile([128, 54], F32, name='ptw')
                idq = identf[32 * q:32 * q + 27, 32 * q:32 * q + 27]
                nc.tensor.transpose(pt[:, 0:27], W0[32 * q:32 * q + 27, 128 * cc:128 * (cc + 1)],
                                    idq, tile_position=(32 * q, 0))
                nc.tensor.transpose(pt[:, 27:54], W1[32 * q:32 * q + 27, 128 * cc:128 * (cc + 1)],
                                    idq, tile_position=(32 * q, 0))
                ta = sbt.tile([128, 54], F32, tag='tall', name='tall')
                nc.vector.tensor_copy(ta[:], pt[:])
                r0 = sbt.tile([128, 9], F32, tag='r0', name='r0')
                nc.vector.tensor_tensor(r0[:], ta[:, 0:9], ta[:, 18:27], ALU.mult)
                r1 = sbt.tile([128, 9], F32, tag='r1', name='r1')
                nc.vector.tensor_tensor(r1[:], ta[:, 27:36], ta[:, 45:54], ALU.mult)
                nc.vector.tensor_tensor(wp3[:, b, 0, :], r0[:], ta[:, 9:18], ALU.mult)
                nc.vector.tensor_tensor(wp3[:, b, 1, :], r0[:], ta[:, 36:45], ALU.mult)
                nc.vector.tensor_tensor(wp3[:, b, 2, :], r1[:], ta[:, 9:18], ALU.mult)
                nc.vector.tensor_tensor(wp3[:, b, 3, :], r1[:], ta[:, 36:45], ALU.mult)

        # ---- stages 5-8: per (half, tap) gather -> blend -> transpose ----
        # einsum accumulates in PSUM inside the loop (delayed one tap so PE
        # never waits on the PSUM->SBUF staging copies)
        nc.sync.dma_start(dw[:], dwpack[:])
        inap = AP(xq[:].tensor, 0, [[512, 4223], [1, 1024]])
        dw4 = dw[:].rearrange('p (k g o c) -> p k g o c', k=9, g=2, o=2)
        with tc.tile_pool(name='psQ', bufs=2, space='PSUM') as psQ, \
             tc.tile_pool(name='psO', bufs=1, space='PSUM') as psO:
            for hb in range(2):
                po = [psO.tile([128, 512], F32, name=f'po{og}{qq}')
                      for og in range(2) for qq in range(2)]

                def einsum_tap(t, stg3):
                    for og in range(2):
                        for qq in range(2):
                            for cg in range(2):
                                nc.tensor.matmul(
                                    po[2 * og + qq][:], dw4[:, t, cg, og, :],
                                    stg3[:, cg, 512 * qq:512 * (qq + 1)],
                                    start=(t == 0 and cg == 0),
                                    stop=(t == 8 and cg == 1))

                prev = None
                gb = None
                gstart = {t: (t, 1) for t in range(9)}
                for t in range(9):
                    if t in gstart:
                        tb, ntap = gstart[t][0], gstart[t][1]
                        gb = gpool2.tile([128, 8 * ntap, 1024], BF16,
                                         tag=f'gq{ntap}', name=f'gq{ntap}')
                        qn = (9 * hb + tb) % 4
                        c0 = 576 * hb + 64 * t
                        nc.gpsimd.dma_gather(
                            out_ap=gb[:], in_ap=inap,
                            idxs_ap=idx16[:, c0: c0 + 64 * ntap],
                            num_idxs=1024 * ntap, num_idxs_reg=1024 * ntap,
                            elem_size=1024, elem_step=512, queue_num=qn)
                        gt0 = t
                    S = spool.tile([128, 8 * 256], BF16, tag='S', name='S')
                    S3 = S[:].rearrange('p (b n) -> p b n', b=8)
                    for blk in range(8):
                        b = 8 * hb + blk
                        gq4 = gb[:]
                        bl = 8 * (t - gt0) + blk
                        m = sba.tile([128, 256], BF16, tag='m', name='m')
                        m2 = sba.tile([128, 256], BF16, tag='m2', name='m2')
                        a1 = sba.tile([128, 256], BF16, tag='a1', name='a1')
                        a2 = sba.tile([128, 256], BF16, tag='a2', name='a2')
                        nc.vector.tensor_scalar_mul(m[:], gq4[:, bl, 0:256],
                                                    wp3[:, b, 0, t:t + 1])
                        nc.scalar.activation(a1[:], gq4[:, bl, 256:512], ACTF.Identity,
                                             scale=wp3[:, b, 2, t:t + 1])
                        nc.vector.tensor_scalar_mul(m2[:], gq4[:, bl, 512:768],
                                                    wp3[:, b, 1, t:t + 1])
                        nc.scalar.activation(a2[:], gq4[:, bl, 768:1024], ACTF.Identity,
                                             scale=wp3[:, b, 3, t:t + 1])
                        nc.vector.tensor_tensor(m[:], m[:], m2[:], ALU.add)
                        nc.vector.tensor_tensor(a1[:], a1[:], a2[:], ALU.add)
                        nc.vector.tensor_tensor(S3[:, blk, :], m[:], a1[:], ALU.add)
                    stg = stpool.tile([128, 2, 1024], BF16, tag='stg', name='stg')
                    for half in range(2):
                        pq = psQ.tile([128, 1024], BF16, name='pq')
                        for j in range(4):
                            blk = 4 * half + j
                            for cg in range(2):
                                nc.tensor.transpose(
                                    pq[:, 256 * j + 128 * cg: 256 * j + 128 * (cg + 1)],
                                    S3[:, blk, 128 * cg:128 * (cg + 1)], ident[:])
                        pq4 = pq[:].rearrange('p (j g c) -> p j g c', j=4, g=2)
                        for cg in range(2):
                            dstp = stg[:, cg, 512 * half:512 * (half + 1)]
                            if half == 0:
                                nc.scalar.activation(
                                    dstp.rearrange('p (j c) -> p j c', j=4),
                                    pq4[:, :, cg, :], ACTF.Identity)
                            else:
                                nc.vector.tensor_copy(
                                    dstp.rearrange('p (j c) -> p j c', j=4),
                                    pq4[:, :, cg, :])
                    if prev is not None:
                        einsum_tap(prev[0], prev[1])
                    prev = (t, stg[:])
                einsum_tap(prev[0], prev[1])
                # ---- bias + store for this half ----
                for og in range(2):
                    for qq in range(2):
                        q = 2 * hb + qq
                        osb = sbt.tile([128, 512], F32, tag='osb', name='osb')
                        nc.scalar.activation(osb[:], po[2 * og + qq][:], ACTF.Identity,
                                             bias=bout_sb[:, og:og + 1], scale=1.0)
                        nc.sync.dma_start(out[128 * og:128 * (og + 1), 512 * q:512 * (q + 1)],
                                          osb[:])

    nc.compile()
    return nc


def prep_core_inputs(inputs, b, h):
    tf = np.ascontiguousarray(np.asarray(inputs['template_feat'][b], dtype=np.float32))
    sf = np.ascontiguousarray(np.asarray(inputs['search_feat'][b], dtype=np.float32))
    offset_w = np.asarray(inputs['offset_w'], dtype=np.float32)
    offset_b = np.asarray(inputs['offset_b'], dtype=np.float32)
    mask_w = np.asarray(inputs['mask_w'], dtype=np.float32)
    mask_b = np.asarray(inputs['mask_b'], dtype=np.float32)
    deform_w = np.asarray(inputs['deform_w'], dtype=np.float32)
    deform_b = np.asarray(inputs['deform_b'], dtype=np.float32)

    tplp = np.zeros((256, 23, 32), np.float32)
    for j in range(19):
        tplp[:, j] = tf[:, min(max(16 * h - 1 + j, 0), 31)]
    if h == 0:
        tplp[:, 21] = tf[:, 15]
        tplp[:, 22] = tf[:, 16]
    else:
        tplp[:, 19] = tf[:, 15]
        tplp[:, 20] = tf[:, 16]

    srch66 = np.zeros((256, 34, 66), np.float32)
    for i in range(34):
        r = 32 * h - 1 + i
        if 0 <= r <= 63:
            srch66[:, i, 1:65] = sf[:, r]

    # quad layout: xq[r] = [pix(r-65), pix(r-1)]; one idx r fetches
    # rows r, r+1 = [TL, BL, TR, BR] corners (2KB).
    sfp = sf.reshape(256, 4096).T  # [4096 px, 256 ch]
    xquad = np.zeros((4224, 512), np.float32)
    xquad[65:65 + 4096, 0:256] = sfp
    xquad[1:1 + 4096, 256:512] = sfp

    wpack = np.zeros((128, 4, 9, 32), np.float32)
    for g in range(4):
        for t, (ky, kx) in enumerate(TAPS):
            cs = slice(128 * g, 128 * (g + 1))
            wpack[:, g, t, 0:9] = offset_w[0::2, cs, ky + 1, kx + 1].T
            wpack[:, g, t, 9:18] = offset_w[1::2, cs, ky + 1, kx + 1].T
            if ky == 0 and kx == 0:
                wpack[:, g, t, 18:27] = mask_w[:, cs, 0, 0].T
    wk = deform_w.reshape(256, 256, 3, 3)
    dwp = np.zeros((128, 9, 2, 2, 128), np.float32)
    for t in range(9):
        ky, kx = TAPS[t]
        for cg in range(2):
            for og in range(2):
                dwp[:, t, cg, og, :] = wk[128 * og:128 * (og + 1),
                                          128 * cg:128 * (cg + 1), ky + 1, kx + 1].T

    basei = np.zeros((128, 512), np.float32)
    col = np.arange(512)
    for q in range(4):
        for m in range(9):
            basei[32 * q + m] = 32 * h + 8 * q + col // 64 + TAPS[m][0] + 16
            basei[32 * q + 9 + m] = col % 64 + TAPS[m][1] + 16

    mcomb = np.zeros((128, 4, 9), np.float32)
    for q in range(4):
        for t in range(9):
            mcomb[32 * q + t, q, t] = 64.0
            mcomb[32 * q + 9 + t, q, t] = 1.0
    bias_comb = np.full((9, 1), float(C0), np.float32)

    bias_om = np.zeros((128, 1), np.float32)
    for q in range(4):
        bias_om[32 * q + 0:32 * q + 9, 0] = offset_b[0::2]
        bias_om[32 * q + 9:32 * q + 18, 0] = offset_b[1::2]
        bias_om[32 * q + 18:32 * q + 27, 0] = mask_b

    rmaski = np.zeros((128, 2), np.float32)
    for q in range(4):
        rmaski[32 * q:32 * q + 18, 0] = 1.0
        rmaski[32 * q + 18:32 * q + 32, 1] = 1.0

    return {
        'rmaski': rmaski,
        'tplp': tplp.reshape(256, 23 * 32).astype(BF16NP),
        'srch66': srch66.reshape(256, 34 * 66).astype(BF16NP),
        'xq': xquad.astype(BF16NP),
        'wpack': wpack.reshape(128, 4 * 9 * 32).astype(BF16NP),
        'dwpack': dwp.reshape(128, 9 * 2 * 2 * 128).astype(BF16NP),
        'basei': basei,
        'mcomb': mcomb.reshape(128, 4 * 9),
        'bias_comb': bias_comb,
        'bias_om': bias_om,
        'bias_out': deform_b.reshape(256, 1).astype(np.float32),
    }


def kernel(**inputs):
    key = 'v2'
    if key not in _NC_CACHE:
        _NC_CACHE[key] = build_nc()
    nc = _NC_CACHE[key]
    in_maps = [prep_core_inputs(inputs, ci // 2, ci % 2) for ci in range(8)]
    res = run_bass_kernel_spmd(nc, in_maps, core_ids=list(range(8)))
    global LAST_RESULT
    LAST_RESULT = res
    out = np.zeros((4, 256, 64, 64), np.float32)
    for ci in range(8):
        b, h = ci // 2, ci % 2
        out[b][:, 32 * h:32 * h + 32, :] = res.results[ci]['out'].reshape(256, 32, 64)
    return out


# revision 37
# speedup vs baseline: 1.0741x; 1.0070x over previous
"""Self-contained Trainium2 kernel for nn_DynamicCrossAttention_40286793236903.

kernel(**inputs) takes the FULL inputs (as produced by setup_inputs) and
returns the FULL [4, 256, 64, 64] float32 output.

Sharding: pure data parallel over (batch, image-half): core ci handles
sample b=ci//2, output rows 32*(ci%2)..32*(ci%2)+31. One SPMD Bass program
runs on all 8 cores; all per-core variation is carried in the input data.

Pipeline per core (all feature data bf16 on device):
  1. upsample template 32x32 -> 64x64 (half-pixel bilinear) and build the
     padded combined tensor [512ch, 34, 66] together with the search half.
  2. offsets+mask 3x3 conv as 36 accumulating PE matmuls per col-quarter.
  3. fp32 index math: sample coords, floor/frac, validity-masked bilinear
     weights; a single clamp serves both the y and x gather coordinate.
  4. gather index assembly (PE transposes) into the 16-partition wrapped
     i16 layout dma_gather wants; one index per (tap, pixel) addresses all
     four bilinear corners through the host-staged quad layout
     xq[r] = [pix(r-65), pix(r-1)]  (2KB per descriptor).
  5. per (half, tap): one SWDGE dma_gather (1024 idxs x 2KB), then a fused
     4-op DVE blend chain per 128-pixel block, PE transpose to channel
     major, and a per-half einsum against the deform weights overlapping
     the other half's loop.
"""
import numpy as np
from contextlib import ExitStack

import ml_dtypes
import concourse.bass as bass
import concourse.mybir as mybir
import concourse.tile as tile
from concourse import bacc
from concourse.bass import AP
from concourse.bass_utils import run_bass_kernel_spmd
from concourse.masks import make_identity

F32 = mybir.dt.float32
BF16 = mybir.dt.bfloat16
I32 = mybir.dt.int32
I16 = mybir.dt.int16
ALU = mybir.AluOpType
ACTF = mybir.ActivationFunctionType
BF16NP = ml_dtypes.bfloat16

TAPS = [(ky, kx) for ky in (-1, 0, 1) for kx in (-1, 0, 1)]
C0 = -(15 * 64) - 16 + 1  # idx = 64*Y + X + C0 -> quad row (see xq layout)

_NC_CACHE = {}
LAST_RESULT = None


def build_nc():
    nc = bacc.Bacc(None, target_bir_lowering=False, num_swdge_queues=4)

    tplp = nc.dram_tensor('tplp', [256, 23 * 32], BF16, kind='ExternalInput')
    srch66 = nc.dram_tensor('srch66', [256, 34 * 66], BF16, kind='ExternalInput')
    xq = nc.dram_tensor('xq', [4224, 512], BF16, kind='ExternalInput')
    wpack = nc.dram_tensor('wpack', [128, 4 * 9 * 32], BF16, kind='ExternalInput')
    dwpack = nc.dram_tensor('dwpack', [128, 9 * 2 * 2 * 128], BF16, kind='ExternalInput')
    basei = nc.dram_tensor('basei', [128, 512], F32, kind='ExternalInput')
    mcomb = nc.dram_tensor('mcomb', [128, 4 * 9], F32, kind='ExternalInput')
    bias_comb = nc.dram_tensor('bias_comb', [9, 1], F32, kind='ExternalInput')
    bias_om = nc.dram_tensor('bias_om', [128, 1], F32, kind='ExternalInput')
    bias_out = nc.dram_tensor('bias_out', [256, 1], F32, kind='ExternalInput')
    rmaski = nc.dram_tensor('rmaski', [128, 2], F32, kind='ExternalInput')
    out = nc.dram_tensor('out', [256, 2048], F32, kind='ExternalOutput')

    with tile.TileContext(nc) as tc, ExitStack() as ctx:
        sb = ctx.enter_context(tc.tile_pool(name='sb', bufs=1))
        sbm = ctx.enter_context(tc.tile_pool(name='sbm', bufs=1))
        sbt = ctx.enter_context(tc.tile_pool(name='sbt', bufs=2))
        sba = ctx.enter_context(tc.tile_pool(name='sba', bufs=3))
        gpool2 = ctx.enter_context(tc.tile_pool(name='gpool2', bufs=6))
        spool = ctx.enter_context(tc.tile_pool(name='spool', bufs=2))
        stpool = ctx.enter_context(tc.tile_pool(name='stpool', bufs=3))

        ident = sb.tile([128, 128], BF16, tag='ident', name='ident')
        make_identity(nc, ident[:])
        identf = sb.tile([128, 128], F32, tag='identf', name='identf')
        make_identity(nc, identf[:])

        wp = sb.tile([128, 4 * 9 * 32], BF16, tag='wp', name='wp')
        dw = sb.tile([128, 9 * 2 * 2 * 128], BF16, tag='dw', name='dw')
        base_sb = sb.tile([128, 512], F32, tag='base', name='base')
        nc.sync.dma_start(base_sb[:], basei[:])
        mcomb_sb = sb.tile([128, 4 * 9], F32, tag='mcomb', name='mcomb')
        nc.sync.dma_start(mcomb_sb[:], mcomb[:])
        bcomb_sb = sb.tile([9, 1], F32, tag='bcomb', name='bcomb')
        nc.sync.dma_start(bcomb_sb[:], bias_comb[:])
        bom_sb = sb.tile([128, 1], F32, tag='bom', name='bom')
        nc.sync.dma_start(bom_sb[:], bias_om[:])
        bout_sb = sb.tile([128, 2], F32, tag='bout', name='bout')
        nc.sync.dma_start(bout_sb[:], bias_out[:].rearrange('(g p) o -> p (g o)', g=2))
        rmask_sb = sb.tile([128, 2], F32, tag='rmask', name='rmask')
        nc.sync.dma_start(rmask_sb[:], rmaski[:])

        # ---- stage 1: upsample template + build combined [512ch, 34, 66] ----
        with tc.tile_pool(name='convsb', bufs=1) as convsb, \
             tc.tile_pool(name='psA', bufs=1, space='PSUM') as psA:
            tps = []
            for cg in range(2):
                tp = convsb.tile([128, 23 * 32], BF16, tag=f'tp{cg}', name=f'tp{cg}')
                nc.sync.dma_start(tp[:], tplp[128 * cg:128 * (cg + 1), :])
                tps.append(tp)
            scbs = []
            for cg in range(2):
                cb = convsb.tile([128, 34 * 66], BF16, tag=f'comb{cg+2}', name=f'comb{cg+2}')
                nc.sync.dma_start(cb[:], srch66[128 * cg:128 * (cg + 1), :])
                scbs.append(cb)
            nc.sync.dma_start(wp[:], wpack[:])
            comb = []
            for cg in range(2):
                tp = tps[cg]
                tp3 = tp[:].rearrange('p (r w) -> p r w', r=23)
                V = convsb.tile([128, 34 * 32], BF16, tag=f'vt{cg}', name=f'vt{cg}')
                V3 = V[:].rearrange('p (r w) -> p r w', r=34)
                tmp = convsb.tile([128, 34 * 32], BF16, tag=f'ut{cg}', name=f'ut{cg}')
                tmp3 = tmp[:].rearrange('p (r w) -> p r w', r=34)
                # vertical: V[i] = wa*tp[j] + wb*tp[j+1] (ts 4x + tt 2x; no stt)
                nc.vector.tensor_scalar_mul(tmp3[:, 0:16, :], tp3[:, 2:18, :], 0.25)
                nc.scalar.activation(V3[:, 2:34:2, :], tp3[:, 1:17, :], ACTF.Identity, scale=0.75)
                nc.vector.tensor_tensor(V3[:, 2:34:2, :], V3[:, 2:34:2, :], tmp3[:, 0:16, :], ALU.add)
                nc.vector.tensor_scalar_mul(tmp3[:, 0:16, :], tp3[:, 1:17, :], 0.75)
                nc.scalar.activation(V3[:, 1:33:2, :], tp3[:, 0:16, :], ACTF.Identity, scale=0.25)
                nc.vector.tensor_tensor(V3[:, 1:33:2, :], V3[:, 1:33:2, :], tmp3[:, 0:16, :], ALU.add)
                nc.vector.tensor_scalar_mul(tmp3[:, 0:1, :], tp3[:, 20:21, :], 0.25)
                nc.scalar.activation(V3[:, 0:1, :], tp3[:, 19:20, :], ACTF.Identity, scale=0.75)
                nc.vector.tensor_tensor(V3[:, 0:1, :], V3[:, 0:1, :], tmp3[:, 0:1, :], ALU.add)
                nc.vector.tensor_scalar_mul(tmp3[:, 0:1, :], tp3[:, 22:23, :], 0.75)
                nc.scalar.activation(V3[:, 33:34, :], tp3[:, 21:22, :], ACTF.Identity, scale=0.25)
                nc.vector.tensor_tensor(V3[:, 33:34, :], V3[:, 33:34, :], tmp3[:, 0:1, :], ALU.add)
                cb = convsb.tile([128, 34 * 66], BF16, tag=f'comb{cg}', name=f'comb{cg}')
                cb3 = cb[:].rearrange('p (r w) -> p r w', r=34)
                nc.vector.memset(cb[:], 0.0)
                h3 = tmp3
                nc.vector.tensor_scalar_mul(h3[:, :, 0:31], V3[:, :, 1:32], 0.75)
                nc.scalar.activation(cb3[:, :, 3:65:2], V3[:, :, 0:31], ACTF.Identity, scale=0.25)
                nc.vector.tensor_tensor(cb3[:, :, 3:65:2], cb3[:, :, 3:65:2], h3[:, :, 0:31], ALU.add)
                nc.vector.tensor_scalar_mul(h3[:, :, 0:31], V3[:, :, 1:32], 0.25)
                nc.scalar.activation(cb3[:, :, 2:64:2], V3[:, :, 0:31], ACTF.Identity, scale=0.75)
                nc.vector.tensor_tensor(cb3[:, :, 2:64:2], cb3[:, :, 2:64:2], h3[:, :, 0:31], ALU.add)
                nc.vector.tensor_copy(cb3[:, :, 1:2], V3[:, :, 0:1])
                nc.vector.tensor_copy(cb3[:, :, 64:65], V3[:, :, 31:32])
                comb.append(cb)
            comb += scbs

            # ---- stage 2: offsets+mask conv, col-tiled quarters ----
            wp4 = wp[:].rearrange('p (g t m) -> p g t m', g=4, t=9)
            pom = psA.tile([128, 512], F32, name='pom')
            for q in range(4):
                first = True
                for gi, g in enumerate((2, 3, 0, 1)):
                    cb3 = comb[g][:].rearrange('p (r w) -> p r w', r=34)
                    for t, (ky, kx) in enumerate(TAPS):
                        rhs = cb3[:, 8 * q + 1 + ky: 8 * q + 9 + ky, 1 + kx: 65 + kx]
                        nc.tensor.matmul(
                            pom[32 * q:32 * q + 32, :], wp4[:, g, t, :], rhs,
                            start=first, stop=(gi == 3 and t == 8),
                            tile_position=(0, 32 * q))
                        first = False
            om = sb.tile([128, 512], F32, tag='om', name='om')
            nc.scalar.activation(om[:], pom[:], ACTF.Identity, bias=bom_sb[:], scale=1.0)

        sg = sb.tile([128, 512], F32, tag='sg', name='sg')
        nc.scalar.activation(sg[:], om[:], ACTF.Sigmoid)

        # ---- stage 3: index math (fp32, in-place tile reuse) ----
        def mtile(tag, dt=F32):
            return sbm.tile([128, 512], dt, tag=tag, name=tag)
        P = mtile('P')          # becomes Wf
        nc.vector.tensor_tensor(P[:], om[:], base_sb[:], ALU.add)
        nc.vector.tensor_scalar(P[:], P[:], 96.5, 14.0, ALU.min, ALU.max)
        T32 = mtile('T32', I32)
        nc.vector.tensor_copy(T32[:], P[:])
        Tf = mtile('Tf')        # becomes F (floor)
        nc.vector.tensor_copy(Tf[:], T32[:])
        Gg = mtile('Gg')        # becomes V0
        nc.vector.tensor_tensor(Gg[:], Tf[:], P[:], ALU.is_gt)
        nc.vector.tensor_tensor(Tf[:], Tf[:], Gg[:], ALU.subtract)
        nc.vector.tensor_tensor(P[:], P[:], Tf[:], ALU.subtract)
        Ff, Wf = Tf, P
        Vt = mtile('Vt')
        V0 = Gg
        nc.vector.tensor_scalar(V0[:], Ff[:], 16.0, None, ALU.is_ge)
        nc.vector.tensor_scalar(Vt[:], Ff[:], 80.0, None, ALU.is_lt)
        nc.vector.tensor_tensor(V0[:], V0[:], Vt[:], ALU.mult)
        V1 = mtile('V1')
        nc.vector.tensor_scalar(V1[:], Ff[:], 15.0, None, ALU.is_ge)
        nc.vector.tensor_scalar(Vt[:], Ff[:], 79.0, None, ALU.is_lt)
        nc.vector.tensor_tensor(V1[:], V1[:], Vt[:], ALU.mult)
        W0 = mtile('W0')
        nc.vector.tensor_scalar(W0[:], Wf[:], -1.0, 1.0, ALU.mult, ALU.add)
        nc.vector.tensor_tensor(W0[:], W0[:], V0[:], ALU.mult)
        W1 = Wf
        nc.vector.tensor_tensor(W1[:], Wf[:], V1[:], ALU.mult)
        # single gather coordinate: clamp(floor, 15, 79) serves y and x rows
        AxC = V1
        nc.vector.tensor_scalar(AxC[:], Ff[:], 79.0, 15.0, ALU.min, ALU.max)

        # ---- stage 4: idx assembly -> idxf [9, 2048] f32 (pixel-major) ----
        mc4 = mcomb_sb[:].rearrange('p (q m) -> p q m', q=4)
        idxf = sb.tile([9, 2048], F32, tag='idxf', name='idxf')
        with tc.tile_pool(name='psI', bufs=2, space='PSUM') as psI:
            for q in range(4):
                pidx = psI.tile([9, 512], F32, name='pidx')
                nc.tensor.matmul(pidx[:], mc4[:, q, :], AxC[:], start=True, stop=True)
                nc.scalar.activation(idxf[:, 512 * q:512 * (q + 1)], pidx[:],
                                      ACTF.Identity, bias=bcomb_sb[:], scale=1.0)
        # wrap for dma_gather, grouped so a 3-tap batch has one contiguous
        # idx window: idx16[j, 576*hb + 64*t + 8*bbl + a] = idxf[t, 128*(8*hb+bbl) + 16*a + j]
        idx16 = sb.tile([128, 2 * 9 * 64], I16, tag='idx16', name='idx16')
        tsb = sb.tile([128, 16 * 9], F32, tag='tsb', name='tsb')
        tsb3 = tsb[:].rearrange('p (b t) -> p b t', b=16)
        with tc.tile_pool(name='psT', bufs=2, space='PSUM') as psT:
            for bb in range(16):
                pT = psT.tile([128, 9], F32, name='pT')
                nc.tensor.transpose(pT[:], idxf[:, 128 * bb:128 * (bb + 1)],
                                    identf[0:9, 0:9], tile_position=(0, 0))
                nc.vector.tensor_copy(tsb3[:, bb, :], pT[:])
            for a in range(8):
                pW = psT.tile([16, 144], F32, name='pW')
                nc.tensor.matmul(pW[:], identf[:, 16 * a:16 * (a + 1)], tsb[:],
                                 start=True, stop=True)
                dsta = AP(idx16[:].tensor, idx16[:].offset + a,
                          [[9 * 128, 16], [576, 2], [8, 8], [64, 9]])
                nc.vector.tensor_copy(
                    dsta, pW[:].rearrange('p (h b t) -> p h b t', h=2, b=8))
        nc.sync.dma_start(idx16[16:32, :], idx16[0:16, :])
        nc.sync.dma_start(idx16[32:64, :], idx16[0:32, :])
        nc.sync.dma_start(idx16[64:128, :], idx16[0:64, :])

        # ---- blend weights: mask-select, transpose, products ----
        for Wt in (W0, W1):
            nc.vector.tensor_scalar(Wt[:], Wt[:], rmask_sb[:, 0:1], None, ALU.mult)
            nc.vector.scalar_tensor_tensor(Wt[:], sg[:], rmask_sb[:, 1:2], Wt[:],
                                           ALU.mult, ALU.add)
        wprod = sb.tile([128, 16 * 6 * 9], F32, tag='wprod', name='wprod')
        wp3 = wprod[:].rearrange('p (b s t) -> p b s t', b=16, s=6)
        with tc.tile_pool(name='psW', bufs=2, space='PSUM') as psW:
            for b in range(16):
                q, cc = b // 4, b % 4
                pt = psW.tile([128, 54], F32, name='ptw')
                idq = identf[32 * q:32 * q + 27, 32 * q:32 * q + 27]
                nc.tensor.transpose(pt[:, 0:27], W0[32 * q:32 * q + 27, 128 * cc:128 * (cc + 1)],
                                    idq, tile_position=(32 * q, 0))
                nc.tensor.transpose(pt[:, 27:54], W1[32 * q:32 * q + 27, 128 * cc:128 * (cc + 1)],
                                    idq, tile_position=(32 * q, 0))
                ta = sbt.tile([128, 54], F32, tag='tall', name='tall')
                nc.vector.tensor_copy(ta[:], pt[:])
                r0 = sbt.tile([128, 9], F32, tag='r0', name='r0')
                nc.vector.tensor_tensor(r0[:], ta[:, 0:9], ta[:, 18:27], ALU.mult)
                r1 = sbt.tile([128, 9], F32, tag='r1', name='r1')
                nc.vector.tensor_tensor(r1[:], ta[:, 27:36], ta[:, 45:54], ALU.mult)
                nc.vector.tensor_tensor(wp3[:, b, 0, :], r0[:], ta[:, 9:18], ALU.mult)
                nc.vector.tensor_tensor(wp3[:, b, 1, :], r0[:], ta[:, 36:45], ALU.mult)
                nc.vector.tensor_tensor(wp3[:, b, 2, :], r1[:], ta[:, 9:18], ALU.mult)
                nc.vector.tensor_tensor(wp3[:, b, 3, :], r1[:], ta[:, 36:45], ALU.mult)

        # ---- stages 5-8: per (half, tap) gather -> blend -> transpose ----
        # einsum accumulates in PSUM inside the loop (delayed one tap so PE
        # never waits on the PSUM->SBUF staging copies)
        nc.sync.dma_start(dw[:], dwpack[:])
        inap = AP(xq[:].tensor, 0, [[512, 4223], [1, 1024]])
        dw4 = dw[:].rearrange('p (k g o c) -> p k g o c', k=9, g=2, o=2)
        with tc.tile_pool(name='psQ', bufs=2, space='PSUM') as psQ, \
             tc.tile_pool(name='psO', bufs=1, space='PSUM') as psO:
            for hb in range(2):
                po = [psO.tile([128, 512], F32, name=f'po{og}{qq}')
                      for og in range(2) for qq in range(2)]

                def einsum_tap(t, stg3):
                    for og in range(2):
                        for qq in range(2):
                            for cg in range(2):
                                nc.tensor.matmul(
                                    po[2 * og + qq][:], dw4[:, t, cg, og, :],
                                    stg3[:, cg, 512 * qq:512 * (qq + 1)],
                                    start=(t == 0 and cg == 0),
                                    stop=(t == 8 and cg == 1))

                prev = None
                gb = None
                gstart = {t: (t, 1) for t in range(9)}
                for t in range(9):
                    if t in gstart:
                        tb, ntap = gstart[t][0], gstart[t][1]
                        gb = gpool2.tile([128, 8 * ntap, 1024], BF16,
                                         tag=f'gq{ntap}', name=f'gq{ntap}')
                        qn = (9 * hb + tb) % 4
                        c0 = 576 * hb + 64 * t
                        nc.gpsimd.dma_gather(
                            out_ap=gb[:], in_ap=inap,
                            idxs_ap=idx16[:, c0: c0 + 64 * ntap],
                            num_idxs=1024 * ntap, num_idxs_reg=1024 * ntap,
                            elem_size=1024, elem_step=512, queue_num=qn)
                        gt0 = t
                    S = spool.tile([128, 8 * 256], BF16, tag='S', name='S')
                    S3 = S[:].rearrange('p (b n) -> p b n', b=8)
                    for blk in range(8):
                        b = 8 * hb + blk
                        gq4 = gb[:]
                        bl = 8 * (t - gt0) + blk
                        m = sba.tile([128, 256], BF16, tag='m', name='m')
                        m2 = sba.tile([128, 256], BF16, tag='m2', name='m2')
                        a1 = sba.tile([128, 256], BF16, tag='a1', name='a1')
                        a2 = sba.tile([128, 256], BF16, tag='a2', name='a2')
                        nc.vector.tensor_scalar_mul(m[:], gq4[:, bl, 0:256],
                                                    wp3[:, b, 0, t:t + 1])
                        nc.scalar.activation(a1[:], gq4[:, bl, 256:512], ACTF.Identity,
                                             scale=wp3[:, b, 2, t:t + 1])
                        nc.vector.tensor_scalar_mul(m2[:], gq4[:, bl, 512:768],
                                                    wp3[:, b, 1, t:t + 1])
                        nc.scalar.activation(a2[:], gq4[:, bl, 768:1024], ACTF.Identity,
                                             scale=wp3[:, b, 3, t:t + 1])
                        nc.vector.tensor_tensor(m[:], m[:], m2[:], ALU.add)
                        nc.vector.tensor_tensor(a1[:], a1[:], a2[:], ALU.add)
                        nc.vector.tensor_tensor(S3[:, blk, :], m[:], a1[:], ALU.add)
                    stg = stpool.tile([128, 2, 1024], BF16, tag='stg', name='stg')
                    for half in range(2):
                        pq = psQ.tile([128, 1024], BF16, name='pq')
                        for j in range(4):
                            blk = 4 * half + j
                            for cg in range(2):
                                nc.tensor.transpose(
                                    pq[:, 256 * j + 128 * cg: 256 * j + 128 * (cg + 1)],
                                    S3[:, blk, 128 * cg:128 * (cg + 1)], ident[:])
                        pq4 = pq[:].rearrange('p (j g c) -> p j g c', j=4, g=2)
                        for cg in range(2):
                            dstp = stg[:, cg, 512 * half:512 * (half + 1)]
                            if half == 0:
                                nc.scalar.activation(
                                    dstp.rearrange('p (j c) -> p j c', j=4),
                                    pq4[:, :, cg, :], ACTF.Identity)
                            else:
                                nc.vector.tensor_copy(
                                    dstp.rearrange('p (j c) -> p j c', j=4),
                                    pq4[:, :, cg, :])
                    if prev is not None:
                        einsum_tap(prev[0], prev[1])
                    prev = (t, stg[:])
                einsum_tap(prev[0], prev[1])
                # ---- bias + store for this half ----
                for og in range(2):
                    for qq in range(2):
                        q = 2 * hb + qq
                        osb = sbt.tile([128, 512], F32, tag='osb', name='osb')
                        nc.scalar.activation(osb[:], po[2 * og + qq][:], ACTF.Identity,
                                             bias=bout_sb[:, og:og + 1], scale=1.0)
                        nc.sync.dma_start(out[128 * og:128 * (og + 1), 512 * q:512 * (q + 1)],
                                          osb[:])

    nc.compile()
    return nc


def prep_core_inputs(inputs, b, h):
    tf = np.ascontiguousarray(np.asarray(inputs['template_feat'][b], dtype=np.float32))
    sf = np.ascontiguousarray(np.asarray(inputs['search_feat'][b], dtype=np.float32))
    offset_w = np.asarray(inputs['offset_w'], dtype=np.float32)
    offset_b = np.asarray(inputs['offset_b'], dtype=np.float32)
    mask_w = np.asarray(inputs['mask_w'], dtype=np.float32)
    mask_b = np.asarray(inputs['mask_b'], dtype=np.float32)
    deform_w = np.asarray(inputs['deform_w'], dtype=np.float32)
    deform_b = np.asarray(inputs['deform_b'], dtype=np.float32)

    tplp = np.zeros((256, 23, 32), np.float32)
    for j in range(19):
        tplp[:, j] = tf[:, min(max(16 * h - 1 + j, 0), 31)]
    if h == 0:
        tplp[:, 21] = tf[:, 15]
        tplp[:, 22] = tf[:, 16]
    else:
        tplp[:, 19] = tf[:, 15]
        tplp[:, 20] = tf[:, 16]

    srch66 = np.zeros((256, 34, 66), np.float32)
    for i in range(34):
        r = 32 * h - 1 + i
        if 0 <= r <= 63:
            srch66[:, i, 1:65] = sf[:, r]

    # quad layout: xq[r] = [pix(r-65), pix(r-1)]; one idx r fetches
    # rows r, r+1 = [TL, BL, TR, BR] corners (2KB).
    sfp = sf.reshape(256, 4096).T  # [4096 px, 256 ch]
    xquad = np.zeros((4224, 512), np.float32)
    xquad[65:65 + 4096, 0:256] = sfp
    xquad[1:1 + 4096, 256:512] = sfp

    wpack = np.zeros((128, 4, 9, 32), np.float32)
    for g in range(4):
        for t, (ky, kx) in enumerate(TAPS):
            cs = slice(128 * g, 128 * (g + 1))
            wpack[:, g, t, 0:9] = offset_w[0::2, cs, ky + 1, kx + 1].T
            wpack[:, g, t, 9:18] = offset_w[1::2, cs, ky + 1, kx + 1].T
            if ky == 0 and kx == 0:
                wpack[:, g, t, 18:27] = mask_w[:, cs, 0, 0].T
    wk = deform_w.reshape(256, 256, 3, 3)
    dwp = np.zeros((128, 9, 2, 2, 128), np.float32)
    for t in range(9):
        ky, kx = TAPS[t]
        for cg in range(2):
            for og in range(2):
                dwp[:, t, cg, og, :] = wk[128 * og:128 * (og + 1),
                                          128 * cg:128 * (cg + 1), ky + 1, kx + 1].T

    basei = np.zeros((128, 512), np.float32)
    col = np.arange(512)
    for q in range(4):
        for m in range(9):
            basei[32 * q + m] = 32 * h + 8 * q + col // 64 + TAPS[m][0] + 16
            basei[32 * q + 9 + m] = col % 64 + TAPS[m][1] + 16

    mcomb = np.zeros((128, 4, 9), np.float32)
    for q in range(4):
        for t in range(9):
            mcomb[32 * q + t, q, t] = 64.0
            mcomb[32 * q + 9 + t, q, t] = 1.0
    bias_comb = np.full((9, 1), float(C0), np.float32)

    bias_om = np.zeros((128, 1), np.float32)
    for q in range(4):
        bias_om[32 * q + 0:32 * q + 9, 0] = offset_b[0::2]
        bias_om[32 * q + 9:32 * q + 18, 0] = offset_b[1::2]
        bias_om[32 * q + 18:32 * q + 27, 0] = mask_b

    rmaski = np.zeros((128, 2), np.float32)
    for q in range(4):
        rmaski[32 * q:32 * q + 18, 0] = 1.0
        rmaski[32 * q + 18:32 * q + 32, 1] = 1.0

    return {
        'rmaski': rmaski,
        'tplp': tplp.reshape(256, 23 * 32).astype(BF16NP),
        'srch66': srch66.reshape(256, 34 * 66).astype(BF16NP),
        'xq': xquad.astype(BF16NP),
        'wpack': wpack.reshape(128, 4 * 9 * 32).astype(BF16NP),
        'dwpack': dwp.reshape(128, 9 * 2 * 2 * 128).astype(BF16NP),
        'basei': basei,
        'mcomb': mcomb.reshape(128, 4 * 9),
        'bias_comb': bias_comb,
        'bias_om': bias_om,
        'bias_out': deform_b.reshape(256, 1).astype(np.float32),
    }


def kernel(**inputs):
    key = 'v2'
    if key not in _NC_CACHE:
        _NC_CACHE[key] = build_nc()
    nc = _NC_CACHE[key]
    in_maps = [prep_core_inputs(inputs, ci // 2, ci % 2) for ci in range(8)]
    res = run_bass_kernel_spmd(nc, in_maps, core_ids=list(range(8)))
    global LAST_RESULT
    LAST_RESULT = res
    out = np.zeros((4, 256, 64, 64), np.float32)
    for ci in range(8):
        b, h = ci // 2, ci % 2
        out[b][:, 32 * h:32 * h + 32, :] = res.results[ci]['out'].reshape(256, 32, 64)
    return out


# revision 40
# speedup vs baseline: 1.0836x; 1.0088x over previous
"""Self-contained Trainium2 kernel for nn_DynamicCrossAttention_40286793236903.

kernel(**inputs) takes the FULL inputs (as produced by setup_inputs) and
returns the FULL [4, 256, 64, 64] float32 output.

Sharding: pure data parallel over (batch, image-half): core ci handles
sample b=ci//2, output rows 32*(ci%2)..32*(ci%2)+31. One SPMD Bass program
runs on all 8 cores; all per-core variation is carried in the input data.

Pipeline per core (all feature data bf16 on device):
  1. upsample template 32x32 -> 64x64 (half-pixel bilinear) and build the
     padded combined tensor [512ch, 34, 66] together with the search half.
  2. offsets+mask 3x3 conv as 36 accumulating PE matmuls per col-quarter.
  3. fp32 index math: sample coords, floor/frac, validity-masked bilinear
     weights; a single clamp serves both the y and x gather coordinate.
  4. gather index assembly (PE transposes) into the 16-partition wrapped
     i16 layout dma_gather wants; one index per (tap, pixel) addresses all
     four bilinear corners through the host-staged quad layout
     xq[r] = [pix(r-65), pix(r-1)]  (2KB per descriptor).
  5. per (half, tap): one SWDGE dma_gather (1024 idxs x 2KB), then a fused
     4-op DVE blend chain per 128-pixel block, PE transpose to channel
     major, and a per-half einsum against the deform weights overlapping
     the other half's loop.
"""
import numpy as np
from contextlib import ExitStack

import ml_dtypes
import concourse.bass as bass
import concourse.mybir as mybir
import concourse.tile as tile
from concourse import bacc
from concourse.bass import AP
from concourse.bass_utils import run_bass_kernel_spmd
from concourse.masks import make_identity

F32 = mybir.dt.float32
BF16 = mybir.dt.bfloat16
I32 = mybir.dt.int32
I16 = mybir.dt.int16
ALU = mybir.AluOpType
ACTF = mybir.ActivationFunctionType
BF16NP = ml_dtypes.bfloat16

TAPS = [(ky, kx) for ky in (-1, 0, 1) for kx in (-1, 0, 1)]
C0 = -(15 * 64) - 16 + 1  # idx = 64*Y + X + C0 -> quad row (see xq layout)

_NC_CACHE = {}
LAST_RESULT = None


def build_nc():
    nc = bacc.Bacc(None, target_bir_lowering=False, num_swdge_queues=4)

    tplp = nc.dram_tensor('tplp', [256, 23 * 32], BF16, kind='ExternalInput')
    srch66 = nc.dram_tensor('srch66', [256, 34 * 66], BF16, kind='ExternalInput')
    xq = nc.dram_tensor('xq', [4224, 512], BF16, kind='ExternalInput')
    wpack = nc.dram_tensor('wpack', [128, 4 * 9 * 32], BF16, kind='ExternalInput')
    dwpack = nc.dram_tensor('dwpack', [128, 9 * 2 * 2 * 128], BF16, kind='ExternalInput')
    basei = nc.dram_tensor('basei', [128, 512], F32, kind='ExternalInput')
    mcomb = nc.dram_tensor('mcomb', [128, 4 * 9], F32, kind='ExternalInput')
    bias_comb = nc.dram_tensor('bias_comb', [9, 1], F32, kind='ExternalInput')
    bias_om = nc.dram_tensor('bias_om', [128, 1], F32, kind='ExternalInput')
    bias_out = nc.dram_tensor('bias_out', [256, 1], F32, kind='ExternalInput')
    rmaski = nc.dram_tensor('rmaski', [128, 2], F32, kind='ExternalInput')
    out = nc.dram_tensor('out', [256, 2048], F32, kind='ExternalOutput')

    with tile.TileContext(nc) as tc, ExitStack() as ctx:
        sb = ctx.enter_context(tc.tile_pool(name='sb', bufs=1))
        sbm = ctx.enter_context(tc.tile_pool(name='sbm', bufs=1))
        sbt = ctx.enter_context(tc.tile_pool(name='sbt', bufs=2))
        sba = ctx.enter_context(tc.tile_pool(name='sba', bufs=3))
        gpool2 = ctx.enter_context(tc.tile_pool(name='gpool2', bufs=6))
        spool = ctx.enter_context(tc.tile_pool(name='spool', bufs=2))
        stpool = ctx.enter_context(tc.tile_pool(name='stpool', bufs=3))

        ident = sb.tile([128, 128], BF16, tag='ident', name='ident')
        make_identity(nc, ident[:])
        identf = sb.tile([128, 128], F32, tag='identf', name='identf')
        make_identity(nc, identf[:])

        wp = sb.tile([128, 4 * 9 * 32], BF16, tag='wp', name='wp')
        dw = sb.tile([128, 9 * 2 * 2 * 128], BF16, tag='dw', name='dw')
        base_sb = sb.tile([128, 512], F32, tag='base', name='base')
        nc.sync.dma_start(base_sb[:], basei[:])
        mcomb_sb = sb.tile([128, 4 * 9], F32, tag='mcomb', name='mcomb')
        nc.sync.dma_start(mcomb_sb[:], mcomb[:])
        bcomb_sb = sb.tile([9, 1], F32, tag='bcomb', name='bcomb')
        nc.sync.dma_start(bcomb_sb[:], bias_comb[:])
        bom_sb = sb.tile([128, 1], F32, tag='bom', name='bom')
        nc.sync.dma_start(bom_sb[:], bias_om[:])
        bout_sb = sb.tile([128, 2], F32, tag='bout', name='bout')
        nc.sync.dma_start(bout_sb[:], bias_out[:].rearrange('(g p) o -> p (g o)', g=2))
        rmask_sb = sb.tile([128, 2], F32, tag='rmask', name='rmask')
        nc.sync.dma_start(rmask_sb[:], rmaski[:])

        # ---- stage 1: upsample template + build combined [512ch, 34, 66] ----
        with tc.tile_pool(name='convsb', bufs=1) as convsb, \
             tc.tile_pool(name='psA', bufs=1, space='PSUM') as psA:
            tps = []
            for cg in range(2):
                tp = convsb.tile([128, 23 * 32], BF16, tag=f'tp{cg}', name=f'tp{cg}')
                nc.sync.dma_start(tp[:], tplp[128 * cg:128 * (cg + 1), :])
                tps.append(tp)
            scbs = []
            for cg in range(2):
                cb = convsb.tile([128, 34 * 66], BF16, tag=f'comb{cg+2}', name=f'comb{cg+2}')
                nc.sync.dma_start(cb[:], srch66[128 * cg:128 * (cg + 1), :])
                scbs.append(cb)
            nc.sync.dma_start(wp[:], wpack[:])
            comb = []
            for cg in range(2):
                tp = tps[cg]
                tp3 = tp[:].rearrange('p (r w) -> p r w', r=23)
                V = convsb.tile([128, 34 * 32], BF16, tag=f'vt{cg}', name=f'vt{cg}')
                V3 = V[:].rearrange('p (r w) -> p r w', r=34)
                tmp = convsb.tile([128, 34 * 32], BF16, tag=f'ut{cg}', name=f'ut{cg}')
                tmp3 = tmp[:].rearrange('p (r w) -> p r w', r=34)
                # vertical: V[i] = wa*tp[j] + wb*tp[j+1] (ts 4x + tt 2x; no stt)
                nc.vector.tensor_scalar_mul(tmp3[:, 0:16, :], tp3[:, 2:18, :], 0.25)
                nc.scalar.activation(V3[:, 2:34:2, :], tp3[:, 1:17, :], ACTF.Identity, scale=0.75)
                nc.vector.tensor_tensor(V3[:, 2:34:2, :], V3[:, 2:34:2, :], tmp3[:, 0:16, :], ALU.add)
                nc.vector.tensor_scalar_mul(tmp3[:, 0:16, :], tp3[:, 1:17, :], 0.75)
                nc.scalar.activation(V3[:, 1:33:2, :], tp3[:, 0:16, :], ACTF.Identity, scale=0.25)
                nc.vector.tensor_tensor(V3[:, 1:33:2, :], V3[:, 1:33:2, :], tmp3[:, 0:16, :], ALU.add)
                nc.vector.tensor_scalar_mul(tmp3[:, 0:1, :], tp3[:, 20:21, :], 0.25)
                nc.scalar.activation(V3[:, 0:1, :], tp3[:, 19:20, :], ACTF.Identity, scale=0.75)
                nc.vector.tensor_tensor(V3[:, 0:1, :], V3[:, 0:1, :], tmp3[:, 0:1, :], ALU.add)
                nc.vector.tensor_scalar_mul(tmp3[:, 0:1, :], tp3[:, 22:23, :], 0.75)
                nc.scalar.activation(V3[:, 33:34, :], tp3[:, 21:22, :], ACTF.Identity, scale=0.25)
                nc.vector.tensor_tensor(V3[:, 33:34, :], V3[:, 33:34, :], tmp3[:, 0:1, :], ALU.add)
                cb = convsb.tile([128, 34 * 66], BF16, tag=f'comb{cg}', name=f'comb{cg}')
                cb3 = cb[:].rearrange('p (r w) -> p r w', r=34)
                nc.vector.memset(cb[:], 0.0)
                h3 = tmp3
                nc.vector.tensor_scalar_mul(h3[:, :, 0:31], V3[:, :, 1:32], 0.75)
                nc.scalar.activation(cb3[:, :, 3:65:2], V3[:, :, 0:31], ACTF.Identity, scale=0.25)
                nc.vector.tensor_tensor(cb3[:, :, 3:65:2], cb3[:, :, 3:65:2], h3[:, :, 0:31], ALU.add)
                nc.vector.tensor_scalar_mul(h3[:, :, 0:31], V3[:, :, 1:32], 0.25)
                nc.scalar.activation(cb3[:, :, 2:64:2], V3[:, :, 0:31], ACTF.Identity, scale=0.75)
                nc.vector.tensor_tensor(cb3[:, :, 2:64:2], cb3[:, :, 2:64:2], h3[:, :, 0:31], ALU.add)
                nc.vector.tensor_copy(cb3[:, :, 1:2], V3[:, :, 0:1])
                nc.vector.tensor_copy(cb3[:, :, 64:65], V3[:, :, 31:32])
                comb.append(cb)
            comb += scbs

            # ---- stages 2-4, pipelined per image-half: conv -> index
            # math -> idx assembly -> blend weights, emitted per half so
            # half 0's gathers launch while half 1's head work continues ----
            wp4 = wp[:].rearrange('p (g t m) -> p g t m', g=4, t=9)
            pom = psA.tile([128, 512], F32, name='pom')
            om = sb.tile([128, 512], F32, tag='om', name='om')
            sg = sb.tile([128, 512], F32, tag='sg', name='sg')

            def mtile(tag, dt=F32):
                return sbm.tile([128, 512], dt, tag=tag, name=tag)
            P = mtile('P')          # becomes Wf
            T32 = mtile('T32', I32)
            Tf = mtile('Tf')        # becomes F (floor)
            Gg = mtile('Gg')        # becomes V0
            Vt = mtile('Vt')
            V1 = mtile('V1')
            W0 = mtile('W0')
            mc4 = mcomb_sb[:].rearrange('p (q m) -> p q m', q=4)
            idxf = sb.tile([9, 2048], F32, tag='idxf', name='idxf')
            idx16 = sb.tile([128, 2 * 9 * 64], I16, tag='idx16', name='idx16')
            tsb = sb.tile([128, 16 * 9], F32, tag='tsb', name='tsb')
            tsb3 = tsb[:].rearrange('p (b t) -> p b t', b=16)
            wprod = sb.tile([128, 16 * 6 * 9], F32, tag='wprod', name='wprod')
            wp3 = wprod[:].rearrange('p (b s t) -> p b s t', b=16, s=6)
            with tc.tile_pool(name='psI', bufs=1, space='PSUM') as psI, \
                 tc.tile_pool(name='psT', bufs=2, space='PSUM') as psT, \
                 tc.tile_pool(name='psW', bufs=1, space='PSUM') as psW:
                for hh in range(2):
                    hs = slice(64 * hh, 64 * hh + 64)
                    for q in (2 * hh, 2 * hh + 1):
                        first = True
                        for gi, g in enumerate((2, 3, 0, 1)):
                            cb3 = comb[g][:].rearrange('p (r w) -> p r w', r=34)
                            for t, (ky, kx) in enumerate(TAPS):
                                rhs = cb3[:, 8 * q + 1 + ky: 8 * q + 9 + ky, 1 + kx: 65 + kx]
                                nc.tensor.matmul(
                                    pom[32 * q:32 * q + 32, :], wp4[:, g, t, :], rhs,
                                    start=first, stop=(gi == 3 and t == 8),
                                    tile_position=(0, 32 * q))
                                first = False
                    nc.scalar.activation(om[hs, :], pom[hs, :], ACTF.Identity,
                                         bias=bom_sb[hs, :], scale=1.0)
                    nc.scalar.activation(sg[hs, :], om[hs, :], ACTF.Sigmoid)
                    nc.vector.tensor_tensor(P[hs, :], om[hs, :], base_sb[hs, :], ALU.add)
                    nc.vector.tensor_scalar(P[hs, :], P[hs, :], 96.5, 14.0, ALU.min, ALU.max)
                    nc.vector.tensor_copy(T32[hs, :], P[hs, :])
                    nc.vector.tensor_copy(Tf[hs, :], T32[hs, :])
                    nc.vector.tensor_tensor(Gg[hs, :], Tf[hs, :], P[hs, :], ALU.is_gt)
                    nc.vector.tensor_tensor(Tf[hs, :], Tf[hs, :], Gg[hs, :], ALU.subtract)
                    nc.vector.tensor_tensor(P[hs, :], P[hs, :], Tf[hs, :], ALU.subtract)
                    Ff, Wf = Tf, P
                    V0 = Gg
                    nc.vector.tensor_scalar(V0[hs, :], Ff[hs, :], 16.0, None, ALU.is_ge)
                    nc.vector.tensor_scalar(Vt[hs, :], Ff[hs, :], 80.0, None, ALU.is_lt)
                    nc.vector.tensor_tensor(V0[hs, :], V0[hs, :], Vt[hs, :], ALU.mult)
                    nc.vector.tensor_scalar(V1[hs, :], Ff[hs, :], 15.0, None, ALU.is_ge)
                    nc.vector.tensor_scalar(Vt[hs, :], Ff[hs, :], 79.0, None, ALU.is_lt)
                    nc.vector.tensor_tensor(V1[hs, :], V1[hs, :], Vt[hs, :], ALU.mult)
                    nc.vector.tensor_scalar(W0[hs, :], Wf[hs, :], -1.0, 1.0, ALU.mult, ALU.add)
                    nc.vector.tensor_tensor(W0[hs, :], W0[hs, :], V0[hs, :], ALU.mult)
                    W1 = Wf
                    nc.vector.tensor_tensor(W1[hs, :], Wf[hs, :], V1[hs, :], ALU.mult)
                    AxC = V1
                    nc.vector.tensor_scalar(AxC[hs, :], Ff[hs, :], 79.0, 15.0, ALU.min, ALU.max)
                    for q in (2 * hh, 2 * hh + 1):
                        pidx = psI.tile([9, 512], F32, name='pidx')
                        nc.tensor.matmul(pidx[:], mc4[hs, q, :], AxC[hs, :],
                                         start=True, stop=True)
                        nc.scalar.activation(idxf[:, 512 * q:512 * (q + 1)], pidx[:],
                                             ACTF.Identity, bias=bcomb_sb[:], scale=1.0)
                    # idx16[j, 576*hh + 64*t + 8*bbl + a] = idxf[t, 128*(8*hh+bbl) + 16*a + j]
                    for bb in range(8 * hh, 8 * hh + 8):
                        pT = psT.tile([128, 9], F32, name='pT')
                        nc.tensor.transpose(pT[:], idxf[:, 128 * bb:128 * (bb + 1)],
                                            identf[0:9, 0:9], tile_position=(0, 0))
                        nc.vector.tensor_copy(tsb3[:, bb, :], pT[:])
                    for a in range(8):
                        pW = psT.tile([16, 72], F32, name='pW')
                        nc.tensor.matmul(pW[:], identf[:, 16 * a:16 * (a + 1)],
                                         tsb[:, 72 * hh:72 * (hh + 1)],
                                         start=True, stop=True)
                        dsta = AP(idx16[:].tensor, idx16[:].offset + 576 * hh + a,
                                  [[9 * 128, 16], [8, 8], [64, 9]])
                        nc.vector.tensor_copy(
                            dsta, pW[:].rearrange('p (b t) -> p b t', b=8))
                    cs = slice(576 * hh, 576 * (hh + 1))
                    nc.sync.dma_start(idx16[16:32, cs], idx16[0:16, cs])
                    nc.sync.dma_start(idx16[32:64, cs], idx16[0:32, cs])
                    nc.sync.dma_start(idx16[64:128, cs], idx16[0:64, cs])
                    for Wt in (W0, W1):
                        nc.vector.tensor_scalar(Wt[hs, :], Wt[hs, :], rmask_sb[hs, 0:1],
                                                None, ALU.mult)
                        nc.vector.scalar_tensor_tensor(Wt[hs, :], sg[hs, :],
                                                       rmask_sb[hs, 1:2], Wt[hs, :],
                                                       ALU.mult, ALU.add)
                    for b in range(8 * hh, 8 * hh + 8):
                        q, cc = b // 4, b % 4
                        pt = psW.tile([128, 54], F32, name='ptw')
                        idq = identf[32 * q:32 * q + 27, 32 * q:32 * q + 27]
                        nc.tensor.transpose(pt[:, 0:27],
                                            W0[32 * q:32 * q + 27, 128 * cc:128 * (cc + 1)],
                                            idq, tile_position=(32 * q, 0))
                        nc.tensor.transpose(pt[:, 27:54],
                                            W1[32 * q:32 * q + 27, 128 * cc:128 * (cc + 1)],
                                            idq, tile_position=(32 * q, 0))
                        ta = sbt.tile([128, 54], F32, tag='tall', name='tall')
                        nc.vector.tensor_copy(ta[:], pt[:])
                        r0 = sbt.tile([128, 9], F32, tag='r0', name='r0')
                        nc.vector.tensor_tensor(r0[:], ta[:, 0:9], ta[:, 18:27], ALU.mult)
                        r1 = sbt.tile([128, 9], F32, tag='r1', name='r1')
                        nc.vector.tensor_tensor(r1[:], ta[:, 27:36], ta[:, 45:54], ALU.mult)
                        nc.vector.tensor_tensor(wp3[:, b, 0, :], r0[:], ta[:, 9:18], ALU.mult)
                        nc.vector.tensor_tensor(wp3[:, b, 1, :], r0[:], ta[:, 36:45], ALU.mult)
                        nc.vector.tensor_tensor(wp3[:, b, 2, :], r1[:], ta[:, 9:18], ALU.mult)
                        nc.vector.tensor_tensor(wp3[:, b, 3, :], r1[:], ta[:, 36:45], ALU.mult)

        # ---- stages 5-8: per (half, tap) gather -> blend -> transpose ----
        # einsum accumulates in PSUM inside the loop (delayed one tap so PE
        # never waits on the PSUM->SBUF staging copies)
        nc.sync.dma_start(dw[:], dwpack[:])
        inap = AP(xq[:].tensor, 0, [[512, 4223], [1, 1024]])
        dw4 = dw[:].rearrange('p (k g o c) -> p k g o c', k=9, g=2, o=2)
        with tc.tile_pool(name='psQ', bufs=2, space='PSUM') as psQ, \
             tc.tile_pool(name='psO', bufs=1, space='PSUM') as psO:
            for hb in range(2):
                po = [psO.tile([128, 512], F32, name=f'po{og}{qq}')
                      for og in range(2) for qq in range(2)]

                def einsum_tap(t, stg3):
                    for og in range(2):
                        for qq in range(2):
                            for cg in range(2):
                                nc.tensor.matmul(
                                    po[2 * og + qq][:], dw4[:, t, cg, og, :],
                                    stg3[:, cg, 512 * qq:512 * (qq + 1)],
                                    start=(t == 0 and cg == 0),
                                    stop=(t == 8 and cg == 1))

                prev = None
                gb = None
                gstart = {t: (t, 1) for t in range(9)}
                for t in range(9):
                    if t in gstart:
                        tb, ntap = gstart[t][0], gstart[t][1]
                        gb = gpool2.tile([128, 8 * ntap, 1024], BF16,
                                         tag=f'gq{ntap}', name=f'gq{ntap}')
                        qn = (9 * hb + tb) % 4
                        c0 = 576 * hb + 64 * t
                        nc.gpsimd.dma_gather(
                            out_ap=gb[:], in_ap=inap,
                            idxs_ap=idx16[:, c0: c0 + 64 * ntap],
                            num_idxs=1024 * ntap, num_idxs_reg=1024 * ntap,
                            elem_size=1024, elem_step=512, queue_num=qn)
                        gt0 = t
                    S = spool.tile([128, 8 * 256], BF16, tag='S', name='S')
                    S3 = S[:].rearrange('p (b n) -> p b n', b=8)
                    for blk in range(8):
                        b = 8 * hb + blk
                        gq4 = gb[:]
                        bl = 8 * (t - gt0) + blk
                        m = sba.tile([128, 256], BF16, tag='m', name='m')
                        m2 = sba.tile([128, 256], BF16, tag='m2', name='m2')
                        a1 = sba.tile([128, 256], BF16, tag='a1', name='a1')
                        a2 = sba.tile([128, 256], BF16, tag='a2', name='a2')
                        nc.vector.tensor_scalar_mul(m[:], gq4[:, bl, 0:256],
                                                    wp3[:, b, 0, t:t + 1])
                        nc.scalar.activation(a1[:], gq4[:, bl, 256:512], ACTF.Identity,
                                             scale=wp3[:, b, 2, t:t + 1])
                        nc.vector.tensor_scalar_mul(m2[:], gq4[:, bl, 512:768],
                                                    wp3[:, b, 1, t:t + 1])
                        nc.scalar.activation(a2[:], gq4[:, bl, 768:1024], ACTF.Identity,
                                             scale=wp3[:, b, 3, t:t + 1])
                        nc.vector.tensor_tensor(m[:], m[:], m2[:], ALU.add)
                        nc.vector.tensor_tensor(a1[:], a1[:], a2[:], ALU.add)
                        nc.vector.tensor_tensor(S3[:, blk, :], m[:], a1[:], ALU.add)
                    stg = stpool.tile([128, 2, 1024], BF16, tag='stg', name='stg')
                    for half in range(2):
                        pq = psQ.tile([128, 1024], BF16, name='pq')
                        for j in range(4):
                            blk = 4 * half + j
                            for cg in range(2):
                                nc.tensor.transpose(
                                    pq[:, 256 * j + 128 * cg: 256 * j + 128 * (cg + 1)],
                                    S3[:, blk, 128 * cg:128 * (cg + 1)], ident[:])
                        pq4 = pq[:].rearrange('p (j g c) -> p j g c', j=4, g=2)
                        for cg in range(2):
                            dstp = stg[:, cg, 512 * half:512 * (half + 1)]
                            if half == 0:
                                nc.scalar.activation(
                                    dstp.rearrange('p (j c) -> p j c', j=4),
                                    pq4[:, :, cg, :], ACTF.Identity)
                            else:
                                nc.vector.tensor_copy(
                                    dstp.rearrange('p (j c) -> p j c', j=4),
                                    pq4[:, :, cg, :])
                    if prev is not None:
                        einsum_tap(prev[0], prev[1])
                    prev = (t, stg[:])
                einsum_tap(prev[0], prev[1])
                # ---- bias + store for this half ----
                for og in range(2):
                    for qq in range(2):
                        q = 2 * hb + qq
                        osb = sbt.tile([128, 512], F32, tag='osb', name='osb')
                        nc.scalar.activation(osb[:], po[2 * og + qq][:], ACTF.Identity,
                                             bias=bout_sb[:, og:og + 1], scale=1.0)
                        nc.sync.dma_start(out[128 * og:128 * (og + 1), 512 * q:512 * (q + 1)],
                                          osb[:])

    nc.compile()
    return nc


def prep_core_inputs(inputs, b, h):
    tf = np.ascontiguousarray(np.asarray(inputs['template_feat'][b], dtype=np.float32))
    sf = np.ascontiguousarray(np.asarray(inputs['search_feat'][b], dtype=np.float32))
    offset_w = np.asarray(inputs['offset_w'], dtype=np.float32)
    offset_b = np.asarray(inputs['offset_b'], dtype=np.float32)
    mask_w = np.asarray(inputs['mask_w'], dtype=np.float32)
    mask_b = np.asarray(inputs['mask_b'], dtype=np.float32)
    deform_w = np.asarray(inputs['deform_w'], dtype=np.float32)
    deform_b = np.asarray(inputs['deform_b'], dtype=np.float32)

    tplp = np.zeros((256, 23, 32), np.float32)
    for j in range(19):
        tplp[:, j] = tf[:, min(max(16 * h - 1 + j, 0), 31)]
    if h == 0:
        tplp[:, 21] = tf[:, 15]
        tplp[:, 22] = tf[:, 16]
    else:
        tplp[:, 19] = tf[:, 15]
        tplp[:, 20] = tf[:, 16]

    srch66 = np.zeros((256, 34, 66), np.float32)
    for i in range(34):
        r = 32 * h - 1 + i
        if 0 <= r <= 63:
            srch66[:, i, 1:65] = sf[:, r]

    # quad layout: xq[r] = [pix(r-65), pix(r-1)]; one idx r fetches
    # rows r, r+1 = [TL, BL, TR, BR] corners (2KB).
    sfp = sf.reshape(256, 4096).T  # [4096 px, 256 ch]
    xquad = np.zeros((4224, 512), np.float32)
    xquad[65:65 + 4096, 0:256] = sfp
    xquad[1:1 + 4096, 256:512] = sfp

    wpack = np.zeros((128, 4, 9, 32), np.float32)
    for g in range(4):
        for t, (ky, kx) in enumerate(TAPS):
            cs = slice(128 * g, 128 * (g + 1))
            wpack[:, g, t, 0:9] = offset_w[0::2, cs, ky + 1, kx + 1].T
            wpack[:, g, t, 9:18] = offset_w[1::2, cs, ky + 1, kx + 1].T
            if ky == 0 and kx == 0:
                wpack[:, g, t, 18:27] = mask_w[:, cs, 0, 0].T
    wk = deform_w.reshape(256, 256, 3, 3)
    dwp = np.zeros((128, 9, 2, 2, 128), np.float32)
    for t in range(9):
        ky, kx = TAPS[t]
        for cg in range(2):
            for og in range(2):
                dwp[:, t, cg, og, :] = wk[128 * og:128 * (og + 1),
                                          128 * cg:128 * (cg + 1), ky + 1, kx + 1].T

    basei = np.zeros((128, 512), np.float32)
    col = np.arange(512)
    for q in range(4):
        for m in range(9):
            basei[32 * q + m] = 32 * h + 8 * q + col // 64 + TAPS[m][0] + 16
            basei[32 * q + 9 + m] = col % 64 + TAPS[m][1] + 16

    mcomb = np.zeros((128, 4, 9), np.float32)
    for q in range(4):
        for t in range(9):
            mcomb[32 * q + t, q, t] = 64.0
            mcomb[32 * q + 9 + t, q, t] = 1.0
    bias_comb = np.full((9, 1), float(C0), np.float32)

    bias_om = np.zeros((128, 1), np.float32)
    for q in range(4):
        bias_om[32 * q + 0:32 * q + 9, 0] = offset_b[0::2]
        bias_om[32 * q + 9:32 * q + 18, 0] = offset_b[1::2]
        bias_om[32 * q + 18:32 * q + 27, 0] = mask_b

    rmaski = np.zeros((128, 2), np.float32)
    for q in range(4):
        rmaski[32 * q:32 * q + 18, 0] = 1.0
        rmaski[32 * q + 18:32 * q + 32, 1] = 1.0

    return {
        'rmaski': rmaski,
        'tplp': tplp.reshape(256, 23 * 32).astype(BF16NP),
        'srch66': srch66.reshape(256, 34 * 66).astype(BF16NP),
        'xq': xquad.astype(BF16NP),
        'wpack': wpack.reshape(128, 4 * 9 * 32).astype(BF16NP),
        'dwpack': dwp.reshape(128, 9 * 2 * 2 * 128).astype(BF16NP),
        'basei': basei,
        'mcomb': mcomb.reshape(128, 4 * 9),
        'bias_comb': bias_comb,
        'bias_om': bias_om,
        'bias_out': deform_b.reshape(256, 1).astype(np.float32),
    }


def kernel(**inputs):
    key = 'v2'
    if key not in _NC_CACHE:
        _NC_CACHE[key] = build_nc()
    nc = _NC_CACHE[key]
    in_maps = [prep_core_inputs(inputs, ci // 2, ci % 2) for ci in range(8)]
    res = run_bass_kernel_spmd(nc, in_maps, core_ids=list(range(8)))
    global LAST_RESULT
    LAST_RESULT = res
    out = np.zeros((4, 256, 64, 64), np.float32)
    for ci in range(8):
        b, h = ci // 2, ci % 2
        out[b][:, 32 * h:32 * h + 32, :] = res.results[ci]['out'].reshape(256, 32, 64)
    return out
